# revision 1
# baseline (speedup 1.0000x reference)
"""MetaPathGNN Trainium2 kernel: 8-core SPMD, node-sharded.

Host (untimed): edge filtering/sorting/partitioning, weight folding, layout prep.
Device: feature-major MLP, AllGather of projected messages, dma_gather of source
rows, PE one-hot matmul segment-sum (PSUM accumulation per 128-dst window),
classifier + log_softmax.
"""

import hashlib
import sys

import numpy as np

sys.path.insert(0, "/opt/trn_rl_repo")

import concourse.bass as bass
import concourse.bacc as bacc
import concourse.mybir as mybir
from concourse.bass_utils import run_bass_kernel_spmd
from concourse.tile import TileContext

N = 50000
P = 8
NPC = 6250          # nodes per core
NPP = 6272          # padded: 49 * 128
NT = NPP // 128     # 49 node tiles / dst windows per core
D = 128
H2 = 256
NCLS = 40
REL0, REL1 = 2, 3
HALF = 4 * NPP      # 25088: int16 gather index range split
CHUNK = 1024        # gather chunk (descriptor ring tops out < 2048)

F32 = mybir.dt.float32
BF16 = mybir.dt.bfloat16
I16 = mybir.dt.int16

import os
REPEAT = int(os.environ.get("KREPEAT", "1"))
SKIP_AG = os.environ.get("SKIP_AG") == "1"
SKIP_GATHER = os.environ.get("SKIP_GATHER") == "1"
SKIP_GRAPH = os.environ.get("SKIP_GRAPH") == "1"
_CACHE = {}
LAST_EXEC_NS = None
LAST_RESULTS = None
TRACE = False
TRACE_KW = {}


def _wrap_idx(a):
    """[L] int16 -> [128, L/16] in (s p) wrapped layout, replicated for 8 q7 cores."""
    sb = a.reshape(-1, 16).T.copy()
    return np.tile(sb, (8, 1))


def _prep_edges(edge_index, edge_type):
    """Per (layer, half): uniform-cap window-sorted edge streams.

    Stream = concat over dst-window w of that window's edges, padded per window
    to cap_w (max count over cores) with (src=0, dstloc=-1) null edges; total
    padded to a CHUNK multiple (tail assigned to the last window).
    Returns dict[(layer, half)] -> (L, bounds, per_core list of (srel, dstloc)).
    bounds[w] = start position of window w in the stream (static, shared).
    """
    ei = np.asarray(edge_index)
    et = np.asarray(edge_type)
    dst_all = ei[0].astype(np.int64)
    src_all = ei[1].astype(np.int64)
    out = {}
    for layer, rel in ((0, REL0), (1, REL1)):
        sel = et == rel
        dst = dst_all[sel]
        src = src_all[sel]
        srow = (src // NPC) * NPP + (src % NPC)
        groups = [[[None] * NT for _ in range(P)] for _ in range(2)]
        for c in range(P):
            m = (dst >= c * NPC) & (dst < (c + 1) * NPC)
            d_loc = (dst[m] - c * NPC).astype(np.int64)
            s_row = srow[m]
            for half in (0, 1):
                hm = (s_row < HALF) if half == 0 else (s_row >= HALF)
                sr = s_row[hm] - half * HALF
                dl = d_loc[hm]
                w = dl // 128
                order = np.argsort(w, kind="stable")
                sr, dl, w = sr[order], dl[order], w[order]
                idx = np.searchsorted(w, np.arange(NT + 1))
                for wi in range(NT):
                    groups[half][c][wi] = (sr[idx[wi]:idx[wi + 1]],
                                           dl[idx[wi]:idx[wi + 1]])
        for half in (0, 1):
            caps = [max(len(groups[half][c][w][0]) for c in range(P))
                    for w in range(NT)]
            L = sum(caps)
            Lpad = ((L + CHUNK - 1) // CHUNK) * CHUNK
            caps[-1] += Lpad - L
            bounds = np.concatenate([[0], np.cumsum(caps)])
            lists = []
            for c in range(P):
                srel = np.zeros(Lpad, np.int64)
                dloc = np.full(Lpad, -1, np.int64)
                for w in range(NT):
                    sr, dl = groups[half][c][w]
                    b = bounds[w]
                    srel[b:b + len(sr)] = sr
                    dloc[b:b + len(dl)] = dl
                lists.append((srel, dloc))
            out[(layer, half)] = (Lpad, bounds, lists)
    return out


def _prep_inputs(inputs):
    f = lambda k: np.asarray(inputs[k], dtype=np.float32)
    x = f("x")
    edges = _prep_edges(inputs["edge_index"], inputs["edge_type"])

    w1, b1 = f("mlp_w1"), f("mlp_b1")
    w2, b2 = f("mlp_w2"), f("mlp_b2")
    w3, b3 = f("mlp_w3"), f("mlp_b3")
    w01_0 = f("w0_0") + f("w1_0")
    ball0 = f("b0_0") + f("b1_0") + f("bl_0")
    w01_1 = f("w0_1") + f("w1_1")
    ball1 = f("b0_1") + f("b1_1") + f("bl_1")
    wl0, wl1 = f("wl_0"), f("wl_1")
    fc1s = f("fc1_w")[:D] + f("fc1_w")[D:]
    fc1b = f("fc1_b")
    fc2w, fc2b = f("fc2_w"), f("fc2_b")
    wcat0 = np.concatenate([wl0, w01_0], axis=1)   # [256, 256] -> [m0 | d0]
    wcat1 = np.concatenate([wl1, w01_1], axis=1)   # [128, 256] -> [m1 | d1]

    iota = np.tile(np.arange(128, dtype=np.float32), (128, 1))
    shared = {
        "w1": w1, "w2": w2, "w3": w3,
        "b1": b1.reshape(D, 1), "b2": b2.reshape(D, 1),
        "b3a": b3[:D].reshape(D, 1), "b3b": b3[D:].reshape(D, 1),
        "wl0a": np.ascontiguousarray(wl0[:D]),
        "wl0b": np.ascontiguousarray(wl0[D:]),
        "w01a": np.ascontiguousarray(w01_0[:D]),
        "w01b": np.ascontiguousarray(w01_0[D:]),
        "wl1": wl1, "w011": w01_1,
        "ball0": ball0.reshape(D, 1), "ball1": ball1.reshape(D, 1),
        "fc1s": fc1s, "fc1b": fc1b.reshape(D, 1),
        "fc2w": fc2w, "fc2b": fc2b.reshape(NCLS, 1),
        "ones40": np.ones((NCLS, 1), np.float32), "ones1x40": np.ones((1, NCLS), np.float32),
    }
    meta = {k: (v[0], v[1]) for k, v in edges.items()}
    # enumerate one-hot ops (w, half, tile) exactly as _build does
    sops = {}
    for layer in (0, 1):
        ops = []
        for w in range(NT):
            for half in (0, 1):
                Lpad, bounds, lists = edges[(layer, half)]
                t0 = bounds[w] // 128
                t1 = (bounds[w + 1] - 1) // 128
                for t in range(t0, t1 + 1):
                    ops.append((w, half, t))
        sops[layer] = ops

    in_maps = []
    for c in range(P):
        m = dict(shared)
        xt = np.zeros((D, NPP), np.float32)
        xt[:, :NPC] = x[c * NPC:(c + 1) * NPC].T
        m["xt"] = xt
        for (layer, half), (Lpad, bounds, lists) in edges.items():
            srel, dloc = lists[c]
            m[f"gs{layer}{half}"] = _wrap_idx(srel.astype(np.int16))
        import ml_dtypes
        for layer in (0, 1):
            ops = sops[layer]
            sall = np.zeros((128, len(ops) * 128), ml_dtypes.bfloat16)
            for i, (w, half, t) in enumerate(ops):
                dloc = edges[(layer, half)][2][c][1][t * 128:(t + 1) * 128]
                rel = dloc - 128 * w
                valid = (rel >= 0) & (rel < 128)
                e = np.nonzero(valid)[0]
                sall[e, i * 128 + rel[valid]] = 1.0
            m[f"sall{layer}"] = sall
        in_maps.append(m)
    return in_maps, meta


def _build(meta):
    nc = bacc.Bacc(None, target_bir_lowering=False, num_swdge_queues=4)

    def din(name, shape, dtype=F32):
        return nc.dram_tensor(name, list(shape), dtype, kind="ExternalInput")

    xt_d = din("xt", (D, NPP))
    wd = {}
    for name, shape in [
        ("w1", (D, D)), ("w2", (D, D)), ("w3", (D, H2)),
        ("b1", (D, 1)), ("b2", (D, 1)), ("b3a", (D, 1)), ("b3b", (D, 1)),
        ("wl0a", (D, D)), ("wl0b", (D, D)), ("w01a", (D, D)), ("w01b", (D, D)),
        ("wl1", (D, D)), ("w011", (D, D)),
        ("ball0", (D, 1)), ("ball1", (D, 1)),
        ("fc1s", (D, D)), ("fc1b", (D, 1)),
        ("fc2w", (D, NCLS)), ("fc2b", (NCLS, 1)),
        ("ones40", (NCLS, 1)), ("ones1x40", (1, NCLS)),
    ]:
        wd[name] = din(name, shape)
    idx_d = {}
    for (layer, half), (Lpad, bounds) in meta.items():
        idx_d[(layer, half, "s")] = din(f"gs{layer}{half}", (128, Lpad // 16), I16)
    nops = {}
    for layer in (0, 1):
        ops = []
        for w in range(NT):
            for half in (0, 1):
                Lpad, bounds = meta[(layer, half)]
                t0 = bounds[w] // 128
                t1 = (bounds[w + 1] - 1) // 128
                for t in range(t0, t1 + 1):
                    ops.append((w, half, t))
        nops[layer] = ops
        idx_d[(layer, "sall")] = din(f"sall{layer}", (128, len(ops) * 128), BF16)

    m_own = [nc.dram_tensor(f"m{i}_own", [NPP, D], BF16) for i in range(2)]
    m_full = [
        nc.dram_tensor(f"m{i}_full", [P * NPP, D], BF16, addr_space="Shared")
        for i in range(2)
    ]
    y_d = nc.dram_tensor("y", [NCLS, NPP], F32, kind="ExternalOutput")

    AF = mybir.ActivationFunctionType
    ALU = mybir.AluOpType
    NCH = 13

    def chunks512():
        for i in range(NCH):
            lo = i * 512
            yield lo, min(512, NPP - lo)

    with TileContext(nc) as tc:
        with tc.tile_pool(name="const", bufs=1) as cpool:
            W = {}
            for name, t in wd.items():
                W[name] = cpool.tile(list(t.shape),
                                     I16 if name.startswith("gs") else F32,
                                     tag=name, name=f"W_{name}")
                nc.sync.dma_start(out=W[name][:], in_=t[:])

            def body(rep):
              with tc.tile_pool(name=f"persist{rep}", bufs=1) as pp:
                dterm = pp.tile([128, NPP], F32, name="dterm")     # node-major
                out_fm = pp.tile([128, NPP], F32, name="out_fm")   # feature-major

                # ---------------- Phase 1: MLP ----------------
                with (
                    tc.tile_pool(name=f"mlp{rep}", bufs=1) as mp,
                    tc.tile_pool(name=f"mlpc{rep}", bufs=3) as mpc,
                ):
                    xt = mp.tile([D, NPP], F32, name="xt_s")
                    nc.sync.dma_start(out=xt[:], in_=xt_d[:])
                    h3 = [mp.tile([D, NPP], F32, name=f"h3_{j}") for j in range(2)]
                    with tc.tile_pool(name=f"psA{rep}", bufs=2, space="PSUM") as psA:
                        for lo, w in chunks512():
                            ps1 = psA.tile([D, 512], F32, tag="ps1", name="ps1")
                            nc.tensor.matmul(ps1[:, :w], W["w1"][:], xt[:, lo:lo + w])
                            h1 = mpc.tile([D, 512], F32, tag="h1", name="h1")
                            nc.scalar.activation(h1[:, :w], ps1[:, :w], AF.Relu,
                                                 bias=W["b1"][:])
                            ps2 = psA.tile([D, 512], F32, tag="ps2", name="ps2")
                            nc.tensor.matmul(ps2[:, :w], W["w2"][:], h1[:, :w])
                            h2t = mpc.tile([D, 512], F32, tag="h2", name="h2")
                            nc.scalar.activation(h2t[:, :w], ps2[:, :w], AF.Relu,
                                                 bias=W["b2"][:])
                            for j in range(2):
                                ps3 = psA.tile([D, 512], F32, tag="ps3", name="ps3")
                                nc.tensor.matmul(
                                    ps3[:, :w], W["w3"][:, j * D:(j + 1) * D],
                                    h2t[:, :w]
                                )
                                nc.scalar.activation(
                                    h3[j][:, lo:lo + w], ps3[:, :w], AF.Identity,
                                    bias=W["b3a"][:] if j == 0 else W["b3b"][:],
                                )
                    # m0 node-major (for gather rows) + d0 feature-major into dterm
                    with (
                        tc.tile_pool(name=f"md0{rep}", bufs=4) as md0p,
                        tc.tile_pool(name=f"psB{rep}", bufs=3, space="PSUM") as psB,
                    ):
                        for lo, w in chunks512():
                            psd = psB.tile([D, 512], F32, tag="d0ps", name="d0ps")
                            nc.tensor.matmul(psd[:, :w], W["w01a"][:], h3[0][:, lo:lo + w],
                                             start=True, stop=False)
                            nc.tensor.matmul(psd[:, :w], W["w01b"][:], h3[1][:, lo:lo + w],
                                             start=False, stop=True)
                            nc.scalar.activation(dterm[:, lo:lo + w], psd[:, :w],
                                                 AF.Identity, bias=W["ball0"][:])
                        for t0g in range(0, NT, 4):
                            tg = list(range(t0g, min(t0g + 4, NT)))
                            ps = psB.tile([128, 512], F32, tag="m0ps", name="m0ps")
                            for j, t in enumerate(tg):
                                lo = t * 128
                                nc.tensor.matmul(ps[:, j * 128:(j + 1) * 128],
                                                 h3[0][:, lo:lo + 128], W["wl0a"][:],
                                                 start=True, stop=False,
                                                 skip_group_check=True)
                                nc.tensor.matmul(ps[:, j * 128:(j + 1) * 128],
                                                 h3[1][:, lo:lo + 128], W["wl0b"][:],
                                                 start=False, stop=True,
                                                 skip_group_check=True)
                            gw = len(tg) * 128
                            m0t = md0p.tile([128, 512], BF16, tag="m0t", name="m0t")
                            nc.scalar.copy(m0t[:, :gw], ps[:, :gw])
                            for j, t in enumerate(tg):
                                lo = t * 128
                                nc.sync.dma_start(out=m_own[0][lo:lo + 128, :],
                                                  in_=m0t[:, j * 128:(j + 1) * 128])

                def allgather(i):
                    if SKIP_AG:
                        return
                    nc.gpsimd.collective_compute(
                        "AllGather", mybir.AluOpType.bypass,
                        ins=[m_own[i][:]], outs=[m_full[i][:]],
                        replica_groups=[list(range(P))],
                    )

                def graph_layer(layer):
                    """PE one-hot segment sum + relu epilogue -> out_fm."""
                    if SKIP_GRAPH:
                        with tc.tile_pool(name=f"sg{rep}_{layer}", bufs=4) as sgp, \
                             tc.tile_pool(name=f"sgp{rep}_{layer}", bufs=3, space="PSUM") as sgps:
                            for w in range(NT):
                                blk = slice(w * 128, (w + 1) * 128)
                                srel = sgp.tile([128, 128], F32, tag="srel", name="srel")
                                nc.scalar.activation(srel[:], dterm[:, blk], AF.Relu)
                                pt = sgps.tile([128, 128], F32, tag="pt", name="pt")
                                nc.tensor.matmul(pt[:], srel[:], W["ident"][:], is_transpose=True)
                                nc.vector.tensor_copy(out_fm[:, blk], pt[:])
                        return
                    with (
                        tc.tile_pool(name=f"gs{rep}_{layer}", bufs=8) as gp,
                        tc.tile_pool(name=f"gi{rep}_{layer}", bufs=1) as gip,
                        tc.tile_pool(name=f"ps{rep}_{layer}", bufs=4, space="PSUM") as psw,
                        tc.tile_pool(name=f"ep{rep}_{layer}", bufs=4) as ep,
                    ):
                        halves = {}
                        for half in (0, 1):
                            Lpad, bounds = meta[(layer, half)]
                            si = gip.tile([128, Lpad // 16], I16, name=f"si{half}",
                                          tag=f"si{half}")
                            nc.sync.dma_start(out=si[:],
                                              in_=idx_d[(layer, half, "s")][:])
                            halves[half] = (Lpad, bounds, si, None, {})
                        nop = len(nops[layer])
                        sall = gip.tile([128, nop * 128], BF16, name="sall",
                                        tag="sall")
                        nc.sync.dma_start(out=sall[:], in_=idx_d[(layer, "sall")][:])
                        opctr = [0]

                        src_view = [m_full[layer][0:HALF, :],
                                    m_full[layer][HALF:2 * HALF, :]]

                        def get_chunk(half, c):
                            Lpad, bounds, si, dl, bufs = halves[half]
                            if SKIP_GATHER:
                                if "z" not in bufs:
                                    g = CHUNK // 128
                                    bufs["z"] = gp.tile([128, g, D], BF16, tag="gbuf", name="gbz")
                                return bufs["z"]
                            if c not in bufs:
                                g = CHUNK // 128
                                buf = gp.tile([128, g, D], BF16, tag="gbuf",
                                              name=f"gb{half}_{c}")
                                nc.gpsimd.dma_gather(
                                    buf[:], src_view[half],
                                    si[:, c * CHUNK // 16:(c + 1) * CHUNK // 16],
                                    CHUNK, CHUNK, D, queue_num=(2 * c + half) % 4,
                                )
                                bufs[c] = buf
                            return bufs[c]

                        for w0 in range(0, NT, 4):
                            ws = list(range(w0, min(w0 + 4, NT)))
                            pw = psw.tile([128, 512], F32, tag="pw", name="pw")
                            for w in ws:
                                off = (w - w0) * 128
                                ops = []  # (half, tile_idx)
                                for half in (0, 1):
                                    Lpad, bounds, si, dl, bufs = halves[half]
                                    t0 = bounds[w] // 128
                                    t1 = (bounds[w + 1] - 1) // 128
                                    for t in range(t0, t1 + 1):
                                        ops.append((half, t))
                                for i, (half, t) in enumerate(ops):
                                    buf = get_chunk(half, t * 128 // CHUNK)
                                    slot = (t * 128 % CHUNK) // 128
                                    oc = opctr[0]
                                    opctr[0] += 1
                                    nc.tensor.matmul(
                                        pw[:, off:off + 128],
                                        buf[:, slot, :],
                                        sall[:, oc * 128:(oc + 1) * 128],
                                        start=(i == 0), stop=(i == len(ops) - 1),
                                        skip_group_check=True,
                                    )
                            gw = len(ws) * 128
                            blk = slice(w0 * 128, w0 * 128 + gw)
                            sadd = ep.tile([128, 512], F32, tag="sadd", name="sadd")
                            nc.vector.tensor_add(sadd[:, :gw], pw[:, :gw], dterm[:, blk])
                            nc.scalar.activation(out_fm[:, blk], sadd[:, :gw], AF.Relu)

                # ---------------- Layer 0 ----------------
                allgather(0)
                graph_layer(0)
                # m1|d1 from out_fm; overwrite dterm with layer-1 dense term
                with (
                    tc.tile_pool(name=f"md1{rep}", bufs=4) as md1p,
                    tc.tile_pool(name=f"psC{rep}", bufs=3, space="PSUM") as psC,
                ):
                    for lo, w in chunks512():
                        psd = psC.tile([D, 512], F32, tag="d1ps", name="d1ps")
                        nc.tensor.matmul(psd[:, :w], W["w011"][:], out_fm[:, lo:lo + w])
                        nc.scalar.activation(dterm[:, lo:lo + w], psd[:, :w],
                                             AF.Identity, bias=W["ball1"][:])
                    for t0g in range(0, NT, 4):
                        tg = list(range(t0g, min(t0g + 4, NT)))
                        ps = psC.tile([128, 512], F32, tag="m1ps", name="m1ps")
                        for j, t in enumerate(tg):
                            lo = t * 128
                            nc.tensor.matmul(ps[:, j * 128:(j + 1) * 128],
                                             out_fm[:, lo:lo + 128], W["wl1"][:],
                                             skip_group_check=True)
                        gw = len(tg) * 128
                        m1t = md1p.tile([128, 512], BF16, tag="m1t", name="m1t")
                        nc.scalar.copy(m1t[:, :gw], ps[:, :gw])
                        for j, t in enumerate(tg):
                            lo = t * 128
                            nc.sync.dma_start(out=m_own[1][lo:lo + 128, :],
                                              in_=m1t[:, j * 128:(j + 1) * 128])

                # ---------------- Layer 1 ----------------
                allgather(1)
                graph_layer(1)

                # ---------------- Classifier + log_softmax ----------------
                with (
                    tc.tile_pool(name=f"fc{rep}", bufs=4) as fcp,
                    tc.tile_pool(name=f"fcb{rep}", bufs=1) as fcbp,
                    tc.tile_pool(name=f"psD{rep}", bufs=2, space="PSUM") as psD,
                ):
                    tfm = fcbp.tile([128, NPP], F32, name="tfm")
                    for lo, w in chunks512():
                        ps = psD.tile([D, 512], F32, tag="fc1ps", name="fc1ps")
                        nc.tensor.matmul(ps[:, :w], W["fc1s"][:], out_fm[:, lo:lo + w])
                        nc.scalar.activation(tfm[:, lo:lo + w], ps[:, :w], AF.Relu,
                                             bias=W["fc1b"][:])
                    for lo, w in chunks512():
                        ps = psD.tile([NCLS, 512], F32, tag="fc2ps", name="fc2ps")
                        nc.tensor.matmul(ps[:, :w], W["fc2w"][:], tfm[:, lo:lo + w])
                        lg = fcp.tile([NCLS, 512], F32, tag="lg", name="lg")
                        nc.scalar.activation(lg[:, :w], ps[:, :w], AF.Identity,
                                             bias=W["fc2b"][:])
                        ex = fcp.tile([NCLS, 512], F32, tag="ex", name="ex")
                        nc.scalar.activation(ex[:, :w], lg[:, :w], AF.Exp)
                        ps2 = psD.tile([1, 512], F32, tag="seps", name="seps")
                        nc.tensor.matmul(ps2[:, :w], W["ones40"][:], ex[:, :w])
                        lnt = fcp.tile([1, 512], F32, tag="lnt", name="lnt")
                        nc.scalar.activation(lnt[:, :w], ps2[:, :w], AF.Ln)
                        ps3 = psD.tile([NCLS, 512], F32, tag="bcps", name="bcps")
                        nc.tensor.matmul(ps3[:, :w], W["ones1x40"][:], lnt[:, :w])
                        yt = fcp.tile([NCLS, 512], F32, tag="yt", name="yt")
                        nc.vector.tensor_sub(yt[:, :w], lg[:, :w], ps3[:, :w])
                        nc.sync.dma_start(out=y_d[:, lo:lo + w], in_=yt[:, :w])
            for rep in range(REPEAT):
                body(rep)
    nc.compile()
    return nc


def kernel(**inputs):
    global LAST_EXEC_NS, LAST_RESULTS
    h = hashlib.md5()
    for k in sorted(inputs):
        h.update(np.ascontiguousarray(np.asarray(inputs[k])).tobytes())
    key = f"{REPEAT}{SKIP_AG}{SKIP_GATHER}{SKIP_GRAPH}" + h.hexdigest()
    if key not in _CACHE:
        in_maps, meta = _prep_inputs(inputs)
        nc = _build({k: (v[0], tuple(v[1])) for k, v in meta.items()})
        _CACHE[key] = (nc, in_maps)
    nc, in_maps = _CACHE[key]
    res = run_bass_kernel_spmd(nc, in_maps, list(range(P)), trace=TRACE, **TRACE_KW)
    LAST_EXEC_NS = res.exec_time_ns
    LAST_RESULTS = res
    outs = res.results
    y = np.concatenate([outs[c]["y"][:, :NPC].T for c in range(P)], axis=0)
    return y.astype(np.float32)



# revision 54
# speedup vs baseline: 130.4361x; 130.4361x over previous
"""MetaPathGNN Trainium2 kernel: 8-core SPMD, node-sharded.

Host (untimed): edge filtering/sorting/partitioning, weight folding, layout prep.
Device: feature-major MLP, AllGather of projected messages, dma_gather of source
rows, PE one-hot matmul segment-sum (PSUM accumulation per 128-dst window),
classifier + log_softmax.
"""

import hashlib
import sys

import numpy as np

sys.path.insert(0, "/opt/trn_rl_repo")

import concourse.bass as bass
import concourse.bacc as bacc
import concourse.mybir as mybir
from concourse.bass_utils import run_bass_kernel_spmd
from concourse.tile import TileContext

N = 50000
P = 8
NPC = 6250          # nodes per core
NPP = 6272          # padded: 49 * 128
NT = NPP // 128     # 49 node tiles / dst windows per core
D = 128
H2 = 256
NCLS = 40
REL0, REL1 = 2, 3
# messages AllGathered in two row-halves (also keeps gather idx in int16):
# half A = local rows [0, HA), half B = [HA, NPP)
HA = 3200           # 25 tiles
HB = NPP - HA       # 3072: 24 tiles
NTA = HA // 128
CHUNK = 1024        # gather chunk (descriptor ring tops out < 2048)

F32 = mybir.dt.float32
BF16 = mybir.dt.bfloat16
F8E4 = mybir.dt.float8e4
I16 = mybir.dt.int16

import os
REPEAT = int(os.environ.get("KREPEAT", "1"))
SKIP_AG = os.environ.get("SKIP_AG") == "1"
SKIP_GATHER = os.environ.get("SKIP_GATHER") == "1"
SKIP_GRAPH = os.environ.get("SKIP_GRAPH") == "1"
PREFETCH = False
PHASES = 9  # truncate body after phase k (1=MLP,2=+md0+AG0,3=+graph0,4=+md1+AG1,5=+graph1,9=all)
_CACHE = {}
LAST_EXEC_NS = None
LAST_RESULTS = None
TRACE = False
TRACE_KW = {}


def _wrap_idx(a):
    """[L] int16 -> [128, L/16] in (s p) wrapped layout, replicated for 8 q7 cores."""
    sb = a.reshape(-1, 16).T.copy()
    return np.tile(sb, (8, 1))


def _prep_edges(edge_index, edge_type):
    """Per (layer, half): uniform-cap window-sorted edge streams.

    Stream = concat over dst-window w of that window's edges, padded per window
    to cap_w (max count over cores) with (src=0, dstloc=-1) null edges; total
    padded to a CHUNK multiple (tail assigned to the last window).
    Returns dict[(layer, half)] -> (L, bounds, per_core list of (srel, dstloc)).
    bounds[w] = start position of window w in the stream (static, shared).
    """
    ei = np.asarray(edge_index)
    et = np.asarray(edge_type)
    dst_all = ei[0].astype(np.int64)
    src_all = ei[1].astype(np.int64)
    src_rank = src_all // NPC
    src_lr = src_all % NPC          # local row on owning core
    out = {}
    for layer, rel in ((0, REL0), (1, REL1)):
        sel = et == rel
        dst = dst_all[sel]
        rank = src_rank[sel]
        lr = src_lr[sel]
        groups = [[[None] * NT for _ in range(P)] for _ in range(2)]
        for c in range(P):
            m = (dst >= c * NPC) & (dst < (c + 1) * NPC)
            d_loc = (dst[m] - c * NPC).astype(np.int64)
            rk = rank[m]
            lrm = lr[m]
            for half in (0, 1):
                hm = (lrm < HA) if half == 0 else (lrm >= HA)
                sr = (rk[hm] * HA + lrm[hm] if half == 0
                      else rk[hm] * HB + (lrm[hm] - HA))
                dl = d_loc[hm]
                w = dl // 128
                order = np.argsort(w, kind="stable")
                sr, dl, w = sr[order], dl[order], w[order]
                idx = np.searchsorted(w, np.arange(NT + 1))
                for wi in range(NT):
                    groups[half][c][wi] = (sr[idx[wi]:idx[wi + 1]],
                                           dl[idx[wi]:idx[wi + 1]])
        for half in (0, 1):
            caps = [max(len(groups[half][c][w][0]) for c in range(P))
                    for w in range(NT)]
            L = sum(caps)
            Lpad = ((L + CHUNK - 1) // CHUNK) * CHUNK
            caps[-1] += Lpad - L
            bounds = np.concatenate([[0], np.cumsum(caps)])
            lists = []
            for c in range(P):
                srel = np.zeros(Lpad, np.int64)
                dloc = np.full(Lpad, -1, np.int64)
                for w in range(NT):
                    sr, dl = groups[half][c][w]
                    b = bounds[w]
                    srel[b:b + len(sr)] = sr
                    dloc[b:b + len(dl)] = dl
                lists.append((srel, dloc))
            out[(layer, half)] = (Lpad, bounds, lists)
    return out


def _prep_inputs(inputs):
    import ml_dtypes
    f = lambda k: np.asarray(inputs[k], dtype=np.float32)
    bf = lambda a: np.ascontiguousarray(a).astype(ml_dtypes.bfloat16)
    x = f("x")
    edges = _prep_edges(inputs["edge_index"], inputs["edge_type"])

    w1, b1 = f("mlp_w1"), f("mlp_b1")
    w2, b2 = f("mlp_w2"), f("mlp_b2")
    w3, b3 = f("mlp_w3"), f("mlp_b3")
    w01_0 = f("w0_0") + f("w1_0")
    ball0 = f("b0_0") + f("b1_0") + f("bl_0")
    w01_1 = f("w0_1") + f("w1_1")
    ball1 = f("b0_1") + f("b1_1") + f("bl_1")
    wl0, wl1 = f("wl_0"), f("wl_1")
    fc1s = f("fc1_w")[:D] + f("fc1_w")[D:]
    fc1b = f("fc1_b")
    fc2w, fc2b = f("fc2_w"), f("fc2_b")
    wcat0 = np.concatenate([wl0, w01_0], axis=1)   # [256, 256] -> [m0 | d0]
    wcat1 = np.concatenate([wl1, w01_1], axis=1)   # [128, 256] -> [m1 | d1]

    shared = {
        "w1": bf(w1), "w2": bf(w2), "w3": bf(w3),
        "b1": b1.reshape(D, 1), "b2": b2.reshape(D, 1),
        "b3a": b3[:D].reshape(D, 1), "b3b": b3[D:].reshape(D, 1),
        "wl0a": bf(wl0[:D]),
        "wl0b": bf(wl0[D:]),
        "w01a": bf(w01_0[:D]),
        "w01b": bf(w01_0[D:]),
        "wl1": bf(wl1), "w011": bf(w01_1),
        "ball0": ball0.reshape(D, 1), "ball1": ball1.reshape(D, 1),
        "fc1s": bf(fc1s), "fc1b": fc1b.reshape(D, 1),
        "fc2w": bf(fc2w),
        "fc2brep": np.tile(fc2b.reshape(1, NCLS), (128, 12)).astype(np.float32),
    }
    meta = {k: (v[0], v[1]) for k, v in edges.items()}
    # enumerate one-hot ops (w, half, tile) exactly as _build does
    sops = {}
    for layer in (0, 1):
        ops = []
        for w in range(NT):
            for half in (0, 1):
                Lpad, bounds, lists = edges[(layer, half)]
                t0 = bounds[w] // 128
                t1 = (bounds[w + 1] - 1) // 128
                for t in range(t0, t1 + 1):
                    ops.append((w, half, t))
        sops[layer] = ops

    in_maps = []
    for c in range(P):
        m = dict(shared)
        xt = np.zeros((D, NPP), np.float32)
        xt[:, :NPC] = x[c * NPC:(c + 1) * NPC].T
        m["xt"] = bf(xt)
        for (layer, half), (Lpad, bounds, lists) in edges.items():
            srel, dloc = lists[c]
            m[f"gs{layer}{half}"] = _wrap_idx(srel.astype(np.int16))
        import ml_dtypes
        for layer in (0, 1):
            ops = sops[layer]
            sall = np.zeros((128, len(ops) * 128), ml_dtypes.float8_e4m3)
            for i, (w, half, t) in enumerate(ops):
                dloc = edges[(layer, half)][2][c][1][t * 128:(t + 1) * 128]
                rel = dloc - 128 * w
                valid = (rel >= 0) & (rel < 128)
                e = np.nonzero(valid)[0]
                sall[e, i * 128 + rel[valid]] = 1.0
            m[f"sall{layer}"] = sall
        in_maps.append(m)
    return in_maps, meta


def _build(meta):
    nc = bacc.Bacc(None, target_bir_lowering=False, num_swdge_queues=4)

    def din(name, shape, dtype=F32):
        return nc.dram_tensor(name, list(shape), dtype, kind="ExternalInput")

    BF_W = {"w1", "w2", "w3", "wl0a", "wl0b", "w01a", "w01b", "wl1", "w011",
            "fc1s", "fc2w"}
    xt_d = din("xt", (D, NPP), BF16)
    wd = {}
    for name, shape in [
        ("w1", (D, D)), ("w2", (D, D)), ("w3", (D, H2)),
        ("b1", (D, 1)), ("b2", (D, 1)), ("b3a", (D, 1)), ("b3b", (D, 1)),
        ("wl0a", (D, D)), ("wl0b", (D, D)), ("w01a", (D, D)), ("w01b", (D, D)),
        ("wl1", (D, D)), ("w011", (D, D)),
        ("ball0", (D, 1)), ("ball1", (D, 1)),
        ("fc1s", (D, D)), ("fc1b", (D, 1)),
        ("fc2w", (D, NCLS)), ("fc2brep", (128, 12 * NCLS)),
    ]:
        wd[name] = din(name, shape, BF16 if name in BF_W else F32)
    idx_d = {}
    for (layer, half), (Lpad, bounds) in meta.items():
        idx_d[(layer, half, "s")] = din(f"gs{layer}{half}", (128, Lpad // 16), I16)
    nops = {}
    for layer in (0, 1):
        ops = []
        for w in range(NT):
            for half in (0, 1):
                Lpad, bounds = meta[(layer, half)]
                t0 = bounds[w] // 128
                t1 = (bounds[w + 1] - 1) // 128
                for t in range(t0, t1 + 1):
                    ops.append((w, half, t))
        nops[layer] = ops
        idx_d[(layer, "sall")] = din(f"sall{layer}", (128, len(ops) * 128), F8E4)

    m_own = [(nc.dram_tensor(f"m{i}_own_a", [HA, D], BF16),
              nc.dram_tensor(f"m{i}_own_b", [HB, D], BF16)) for i in range(2)]
    m_full = [(nc.dram_tensor(f"m{i}_full_a", [P * HA, D], BF16,
                              addr_space="Shared"),
               nc.dram_tensor(f"m{i}_full_b", [P * HB, D], BF16,
                              addr_space="Shared")) for i in range(2)]

    def m_own_dst(i, t):
        """DRAM row-slice of m{i}_own_{a,b} for local node tile t."""
        if t < NTA:
            return m_own[i][0][t * 128:(t + 1) * 128, :]
        return m_own[i][1][(t - NTA) * 128:(t - NTA + 1) * 128, :]
    # node-major tiles dumped partition-major: y[p, t*NCLS+c] = node t*128+p
    y_d = nc.dram_tensor("y", [128, NT * NCLS], F32, kind="ExternalOutput")

    AF = mybir.ActivationFunctionType
    ALU = mybir.AluOpType
    NCH = 13

    def chunks512():
        for i in range(NCH):
            lo = i * 512
            yield lo, min(512, NPP - lo)

    with TileContext(nc) as tc:
        with tc.tile_pool(name="const", bufs=1) as cpool:
            W = {}
            for name, t in wd.items():
                W[name] = cpool.tile(list(t.shape), t.dtype,
                                     tag=name, name=f"W_{name}")
                nc.sync.dma_start(out=W[name][:], in_=t[:])

            def body(rep):
              from contextlib import ExitStack
              with tc.tile_pool(name=f"persist{rep}", bufs=1) as pp:
                dterm = pp.tile([128, NPP], F32, name="dterm")     # node-major
                out_fm = pp.tile([128, NPP], BF16, name="out_fm")  # feature-major

                def load_graph_inputs(layer, gip, npieces=8):
                    """si/sall loads on the ACT HWDGE ring (overlaps other DMA).
                    sall is loaded in pieces so early consumers unblock early."""
                    pre = {}
                    for half in (0, 1):
                        Lpad, bounds = meta[(layer, half)]
                        si = gip.tile([128, Lpad // 16], I16,
                                      name=f"si{layer}_{half}", tag=f"si{half}")
                        nc.scalar.dma_start(out=si[:],
                                            in_=idx_d[(layer, half, "s")][:])
                        pre[half] = si
                    pre["sall"] = load_sall(layer, gip, npieces)
                    return pre

                def load_sall(layer, gip, npieces=8):
                    nop = len(nops[layer])
                    sall = gip.tile([128, nop * 128], F8E4, name=f"sall{layer}",
                                    tag="sall")
                    step = (nop + npieces - 1) // npieces * 128
                    for lo in range(0, nop * 128, step):
                        hi = min(lo + step, nop * 128)
                        nc.scalar.dma_start(out=sall[:, lo:hi],
                                            in_=idx_d[(layer, "sall")][:, lo:hi])
                    return sall

                # layer-0 graph inputs stream in during the MLP phase;
                # layer-1 si too (small). layer-1 sall streams during md1/AG1.
                stk1si = ExitStack()
                gip1si = stk1si.enter_context(
                    tc.tile_pool(name=f"gi{rep}_1si", bufs=1))
                pre1 = {}
                for half in (0, 1):
                    Lpad, bounds = meta[(1, half)]
                    si = gip1si.tile([128, Lpad // 16], I16,
                                     name=f"si1_{half}", tag=f"si1_{half}")
                    nc.scalar.dma_start(out=si[:], in_=idx_d[(1, half, "s")][:])
                    pre1[half] = si
                stk0 = ExitStack()
                gip0 = stk0.enter_context(tc.tile_pool(name=f"gi{rep}_0", bufs=1))
                pre0 = load_graph_inputs(0, gip0)

                # ---------------- Phase 1: MLP ----------------
                with (
                    tc.tile_pool(name=f"mlp{rep}", bufs=1) as mp,
                    tc.tile_pool(name=f"mlpc{rep}", bufs=3) as mpc,
                ):
                    xt = mp.tile([D, NPP], BF16, name="xt_s")
                    nc.sync.dma_start(out=xt[:], in_=xt_d[:])
                    h3 = [mp.tile([D, NPP], BF16, name=f"h3_{j}") for j in range(2)]
                    with tc.tile_pool(name=f"psA{rep}", bufs=2, space="PSUM") as psA:
                        for lo, w in chunks512():
                            ps1 = psA.tile([D, 512], F32, tag="ps1", name="ps1")
                            nc.tensor.matmul(ps1[:, :w], W["w1"][:], xt[:, lo:lo + w])
                            h1 = mpc.tile([D, 512], BF16, tag="h1", name="h1")
                            nc.scalar.activation(h1[:, :w], ps1[:, :w], AF.Relu,
                                                 bias=W["b1"][:])
                            ps2 = psA.tile([D, 512], F32, tag="ps2", name="ps2")
                            nc.tensor.matmul(ps2[:, :w], W["w2"][:], h1[:, :w])
                            h2t = mpc.tile([D, 512], BF16, tag="h2", name="h2")
                            nc.scalar.activation(h2t[:, :w], ps2[:, :w], AF.Relu,
                                                 bias=W["b2"][:])
                            for j in range(2):
                                ps3 = psA.tile([D, 512], F32, tag="ps3", name="ps3")
                                nc.tensor.matmul(
                                    ps3[:, :w], W["w3"][:, j * D:(j + 1) * D],
                                    h2t[:, :w]
                                )
                                nc.vector.tensor_scalar_add(
                                    h3[j][:, lo:lo + w], ps3[:, :w],
                                    W["b3a"][:] if j == 0 else W["b3b"][:],
                                )
                    # m0 node-major (for gather rows) + d0 feature-major into dterm
                    with (
                        tc.tile_pool(name=f"md0{rep}", bufs=4) as md0p,
                        tc.tile_pool(name=f"psB{rep}", bufs=3, space="PSUM") as psB,
                    ):
                      if PHASES >= 2:
                        for t0g in range(0, NT, 4):
                            tg = list(range(t0g, min(t0g + 4, NT)))
                            ps = psB.tile([128, 512], F32, tag="m0ps", name="m0ps")
                            for j, t in enumerate(tg):
                                lo = t * 128
                                nc.tensor.matmul(ps[:, j * 128:(j + 1) * 128],
                                                 h3[0][:, lo:lo + 128], W["wl0a"][:],
                                                 start=True, stop=False,
                                                 skip_group_check=True)
                                nc.tensor.matmul(ps[:, j * 128:(j + 1) * 128],
                                                 h3[1][:, lo:lo + 128], W["wl0b"][:],
                                                 start=False, stop=True,
                                                 skip_group_check=True)
                            gw = len(tg) * 128
                            m0t = md0p.tile([128, 512], BF16, tag="m0t", name="m0t")
                            nc.vector.tensor_copy(m0t[:, :gw], ps[:, :gw])
                            for j, t in enumerate(tg):
                                nc.sync.dma_start(out=m_own_dst(0, t),
                                                  in_=m0t[:, j * 128:(j + 1) * 128])
                        for lo, w in chunks512():
                            psd = psB.tile([D, 512], F32, tag="d0ps", name="d0ps")
                            nc.tensor.matmul(psd[:, :w], W["w01a"][:], h3[0][:, lo:lo + w],
                                             start=True, stop=False)
                            nc.tensor.matmul(psd[:, :w], W["w01b"][:], h3[1][:, lo:lo + w],
                                             start=False, stop=True)
                            nc.scalar.activation(dterm[:, lo:lo + w], psd[:, :w],
                                                 AF.Identity, bias=W["ball0"][:])
                if PHASES < 2:
                    stk0.close()
                    stk1si.close()
                    return

                def allgather(i):
                    if SKIP_AG:
                        return
                    for h in (0, 1):
                        nc.gpsimd.collective_compute(
                            "AllGather", mybir.AluOpType.bypass,
                            ins=[m_own[i][h][:]], outs=[m_full[i][h][:]],
                            replica_groups=[list(range(P))],
                        )

                def graph_layer(layer, pre=None):
                    """PE one-hot segment sum + relu epilogue -> out_fm."""
                    if SKIP_GRAPH:
                        with tc.tile_pool(name=f"sg{rep}_{layer}", bufs=4) as sgp, \
                             tc.tile_pool(name=f"sgp{rep}_{layer}", bufs=3, space="PSUM") as sgps:
                            for w in range(NT):
                                blk = slice(w * 128, (w + 1) * 128)
                                srel = sgp.tile([128, 128], F32, tag="srel", name="srel")
                                nc.scalar.activation(srel[:], dterm[:, blk], AF.Relu)
                                pt = sgps.tile([128, 128], F32, tag="pt", name="pt")
                                nc.tensor.matmul(pt[:], srel[:], W["ident"][:], is_transpose=True)
                                nc.vector.tensor_copy(out_fm[:, blk], pt[:])
                        return
                    with ExitStack() as lstk:
                        gpa = lstk.enter_context(
                            tc.tile_pool(name=f"gsa{rep}_{layer}", bufs=10))
                        gpb = lstk.enter_context(
                            tc.tile_pool(name=f"gsb{rep}_{layer}", bufs=10))
                        psw = lstk.enter_context(
                            tc.tile_pool(name=f"ps{rep}_{layer}", bufs=6,
                                         space="PSUM"))
                        ep = lstk.enter_context(
                            tc.tile_pool(name=f"ep{rep}_{layer}", bufs=4))
                        if pre is None:
                            gip = lstk.enter_context(
                                tc.tile_pool(name=f"gi{rep}_{layer}", bufs=1))
                            pre = load_graph_inputs(layer, gip)
                        halves = {}
                        for half in (0, 1):
                            Lpad, bounds = meta[(layer, half)]
                            halves[half] = (Lpad, bounds, pre[half], None, {})
                        sall = pre["sall"]
                        opctr = [0]

                        src_view = [m_full[layer][0][:], m_full[layer][1][:]]
                        gpool = {0: gpa, 1: gpb}
                        qctr = [0]

                        def get_chunk(half, c):
                            Lpad, bounds, si, dl, bufs = halves[half]
                            gp = gpool[half]
                            if SKIP_GATHER:
                                if "z" not in bufs:
                                    g = CHUNK // 128
                                    bufs["z"] = gp.tile([128, g, D], BF16, tag="gbuf", name=f"gbz{half}")
                                    nc.vector.memset(bufs["z"][:], 0)
                                return bufs["z"]
                            if c not in bufs:
                                g = CHUNK // 128
                                buf = gp.tile([128, g, D], BF16, tag="gbuf",
                                              name=f"gb{half}_{c}")
                                nc.gpsimd.dma_gather(
                                    buf[:], src_view[half],
                                    si[:, c * CHUNK // 16:(c + 1) * CHUNK // 16],
                                    CHUNK, CHUNK, D, queue_num=qctr[0] % 4,
                                )
                                qctr[0] += 1
                                bufs[c] = buf
                            return bufs[c]

                        # issue all gathers upfront (half A first: its
                        # AllGather completes first, half-A gathers overlap
                        # half B's transfer)
                        if not SKIP_GATHER and PREFETCH:
                            for half in (0, 1):
                                Lpad = halves[half][0]
                                for c in range(Lpad // CHUNK):
                                    get_chunk(half, c)

                        for w0 in range(0, NT, 4):
                            ws = list(range(w0, min(w0 + 4, NT)))
                            pw = psw.tile([128, 512], F32, tag="pw", name="pw")
                            for w in ws:
                                off = (w - w0) * 128
                                ops = []  # (half, tile_idx)
                                for half in (0, 1):
                                    Lpad, bounds, si, dl, bufs = halves[half]
                                    t0 = bounds[w] // 128
                                    t1 = (bounds[w + 1] - 1) // 128
                                    for t in range(t0, t1 + 1):
                                        ops.append((half, t))
                                for i, (half, t) in enumerate(ops):
                                    buf = get_chunk(half, t * 128 // CHUNK)
                                    slot = (t * 128 % CHUNK) // 128
                                    oc = opctr[0]
                                    opctr[0] += 1
                                    nc.tensor.matmul(
                                        pw[:, off:off + 128],
                                        buf[:, slot, :],
                                        sall[:, oc * 128:(oc + 1) * 128],
                                        start=(i == 0), stop=(i == len(ops) - 1),
                                        skip_group_check=True,
                                    )
                            gw = len(ws) * 128
                            blk = slice(w0 * 128, w0 * 128 + gw)
                            sadd = ep.tile([128, 512], F32, tag="sadd", name="sadd")
                            nc.vector.tensor_add(sadd[:, :gw], pw[:, :gw], dterm[:, blk])
                            nc.scalar.activation(out_fm[:, blk], sadd[:, :gw], AF.Relu)

                # ---------------- Layer 0 ----------------
                allgather(0)
                if PHASES < 3:
                    stk0.close()
                    stk1si.close()
                    return
                graph_layer(0, pre0)
                stk0.close()
                if PHASES < 4:
                    stk1si.close()
                    return
                # layer-1 sall streams in during md1 + AG1
                stk1 = ExitStack()
                gip1 = stk1.enter_context(tc.tile_pool(name=f"gi{rep}_1sall", bufs=1))
                pre1["sall"] = load_sall(1, gip1)
                # m1|d1 from out_fm; overwrite dterm with layer-1 dense term
                with (
                    tc.tile_pool(name=f"md1{rep}", bufs=4) as md1p,
                    tc.tile_pool(name=f"psC{rep}", bufs=3, space="PSUM") as psC,
                ):
                    for t0g in range(0, NT, 4):
                        tg = list(range(t0g, min(t0g + 4, NT)))
                        ps = psC.tile([128, 512], F32, tag="m1ps", name="m1ps")
                        for j, t in enumerate(tg):
                            lo = t * 128
                            nc.tensor.matmul(ps[:, j * 128:(j + 1) * 128],
                                             out_fm[:, lo:lo + 128], W["wl1"][:],
                                             skip_group_check=True)
                        gw = len(tg) * 128
                        m1t = md1p.tile([128, 512], BF16, tag="m1t", name="m1t")
                        nc.vector.tensor_copy(m1t[:, :gw], ps[:, :gw])
                        for j, t in enumerate(tg):
                            nc.sync.dma_start(out=m_own_dst(1, t),
                                              in_=m1t[:, j * 128:(j + 1) * 128])
                    for lo, w in chunks512():
                        psd = psC.tile([D, 512], F32, tag="d1ps", name="d1ps")
                        nc.tensor.matmul(psd[:, :w], W["w011"][:], out_fm[:, lo:lo + w])
                        nc.scalar.activation(dterm[:, lo:lo + w], psd[:, :w],
                                             AF.Identity, bias=W["ball1"][:])

                # ---------------- Layer 1 ----------------
                allgather(1)
                if PHASES < 5:
                    stk1.close()
                    stk1si.close()
                    return
                graph_layer(1, pre1)
                if PHASES < 6:
                    stk1.close()
                    stk1si.close()
                    return

                # ---------------- Classifier + log_softmax ----------------
                # fc2/softmax in node-major: nodes on partitions, per-node
                # log-sum-exp via free-dim segmented reduce.
                with (
                    tc.tile_pool(name=f"fc{rep}", bufs=4) as fcp,
                    tc.tile_pool(name=f"fcb{rep}", bufs=1) as fcbp,
                    tc.tile_pool(name=f"psD{rep}", bufs=2, space="PSUM") as psD,
                ):
                    tfm = fcbp.tile([128, NPP], BF16, name="tfm")
                    for lo, w in chunks512():
                        ps = psD.tile([D, 512], F32, tag="fc1ps", name="fc1ps")
                        nc.tensor.matmul(ps[:, :w], W["fc1s"][:], out_fm[:, lo:lo + w])
                        nc.scalar.activation(tfm[:, lo:lo + w], ps[:, :w], AF.Relu,
                                             bias=W["fc1b"][:])
                    for g0 in range(0, NT, 12):
                        tg = list(range(g0, min(g0 + 12, NT)))
                        ng = len(tg)
                        gw = ng * NCLS
                        ps = psD.tile([128, 12 * NCLS], F32, tag="fc2ps", name="fc2ps")
                        for j, t in enumerate(tg):
                            nc.tensor.matmul(ps[:, j * NCLS:(j + 1) * NCLS],
                                             tfm[:, t * 128:(t + 1) * 128],
                                             W["fc2w"][:], skip_group_check=True)
                        lg = fcp.tile([128, 12 * NCLS], F32, tag="lg", name="lg")
                        nc.vector.tensor_add(lg[:, :gw], ps[:, :gw],
                                             W["fc2brep"][:, :gw])
                        ex = fcp.tile([128, 12 * NCLS], BF16, tag="ex", name="ex")
                        nc.scalar.activation(ex[:, :gw], lg[:, :gw], AF.Exp)
                        s = fcp.tile([128, 12], F32, tag="s", name="s")
                        nc.vector.reduce_sum(
                            s[:, :ng],
                            ex[:, :gw].rearrange("p (n c) -> p n c", c=NCLS),
                            axis=mybir.AxisListType.X)
                        lns = fcp.tile([128, 12], F32, tag="lns", name="lns")
                        nc.scalar.activation(lns[:, :ng], s[:, :ng], AF.Ln)
                        yt = fcp.tile([128, 12 * NCLS], F32, tag="yt", name="yt")
                        nc.vector.tensor_sub(
                            yt[:, :gw].rearrange("p (n c) -> p n c", c=NCLS),
                            lg[:, :gw].rearrange("p (n c) -> p n c", c=NCLS),
                            lns[:, :ng].rearrange("p (n o) -> p n o", o=1)
                               .to_broadcast([128, ng, NCLS]))
                        nc.sync.dma_start(
                            out=y_d[:, g0 * NCLS:g0 * NCLS + gw],
                            in_=yt[:, :gw])
                stk1.close()
                stk1si.close()
            for rep in range(REPEAT):
                body(rep)
    nc.compile()
    return nc


def kernel(**inputs):
    global LAST_EXEC_NS, LAST_RESULTS
    h = hashlib.md5()
    for k in sorted(inputs):
        h.update(np.ascontiguousarray(np.asarray(inputs[k])).tobytes())
    key = f"{REPEAT}{SKIP_AG}{SKIP_GATHER}{SKIP_GRAPH}{PREFETCH}{PHASES}" + h.hexdigest()
    if key not in _CACHE:
        in_maps, meta = _prep_inputs(inputs)
        nc = _build({k: (v[0], tuple(v[1])) for k, v in meta.items()})
        _CACHE[key] = (nc, in_maps)
    nc, in_maps = _CACHE[key]
    res = run_bass_kernel_spmd(nc, in_maps, list(range(P)), trace=TRACE, **TRACE_KW)
    LAST_EXEC_NS = res.exec_time_ns
    LAST_RESULTS = res
    outs = res.results
    y = np.concatenate(
        [outs[c]["y"].reshape(128, NT, NCLS).transpose(1, 0, 2)
         .reshape(NPP, NCLS)[:NPC] for c in range(P)],
        axis=0)
    return np.ascontiguousarray(y, dtype=np.float32)



# revision 55
# speedup vs baseline: 130.7355x; 1.0023x over previous
"""MetaPathGNN Trainium2 kernel: 8-core SPMD, node-sharded. ~427us/iter.

Host (untimed): edge filtering/sorting/partitioning, weight folding, layout prep.
Device: bf16 feature-major MLP, AllGather (2 row-halves; the half split doubles
as the int16 gather-index split) of bf16 projected messages, 4-queue SWDGE
dma_gather of per-edge source rows, PE one-hot (fp8e4) matmul segment-sum with
PSUM accumulation per 128-dst window, node-major fc2 + log_softmax via
free-dim segmented reduce. Layer-0 sall/si preloaded during the MLP; layer-1
sall streamed in pieces during md1/AG1.

Measured bottleneck structure (per rep): AllGathers ~130us + gathers ~125us
(4-SWDGE-queue limit, ~2.2ns/row) + compute ~160us; AG/gather overlap attempts
consistently LOST time (shared SDMA/HBM resources) - serial at full rate beats
overlapped at half rate here.
"""

import hashlib
import sys

import numpy as np

sys.path.insert(0, "/opt/trn_rl_repo")

import concourse.bass as bass
import concourse.bacc as bacc
import concourse.mybir as mybir
from concourse.bass_utils import run_bass_kernel_spmd
from concourse.tile import TileContext

N = 50000
P = 8
NPC = 6250          # nodes per core
NPP = 6272          # padded: 49 * 128
NT = NPP // 128     # 49 node tiles / dst windows per core
D = 128
H2 = 256
NCLS = 40
REL0, REL1 = 2, 3
# messages AllGathered in two row-halves (also keeps gather idx in int16):
# half A = local rows [0, HA), half B = [HA, NPP)
HA = 3200           # 25 tiles
HB = NPP - HA       # 3072: 24 tiles
NTA = HA // 128
CHUNK = 1024        # gather chunk (descriptor ring tops out < 2048)

F32 = mybir.dt.float32
BF16 = mybir.dt.bfloat16
F8E4 = mybir.dt.float8e4
I16 = mybir.dt.int16

import os
REPEAT = int(os.environ.get("KREPEAT", "1"))
SKIP_AG = os.environ.get("SKIP_AG") == "1"
SKIP_GATHER = os.environ.get("SKIP_GATHER") == "1"
SKIP_GRAPH = os.environ.get("SKIP_GRAPH") == "1"
PREFETCH = False
PHASES = 9  # truncate body after phase k (1=MLP,2=+md0+AG0,3=+graph0,4=+md1+AG1,5=+graph1,9=all)
_CACHE = {}
LAST_EXEC_NS = None
LAST_RESULTS = None
TRACE = False
TRACE_KW = {}


def _wrap_idx(a):
    """[L] int16 -> [128, L/16] in (s p) wrapped layout, replicated for 8 q7 cores."""
    sb = a.reshape(-1, 16).T.copy()
    return np.tile(sb, (8, 1))


def _prep_edges(edge_index, edge_type):
    """Per (layer, half): uniform-cap window-sorted edge streams.

    Stream = concat over dst-window w of that window's edges, padded per window
    to cap_w (max count over cores) with (src=0, dstloc=-1) null edges; total
    padded to a CHUNK multiple (tail assigned to the last window).
    Returns dict[(layer, half)] -> (L, bounds, per_core list of (srel, dstloc)).
    bounds[w] = start position of window w in the stream (static, shared).
    """
    ei = np.asarray(edge_index)
    et = np.asarray(edge_type)
    dst_all = ei[0].astype(np.int64)
    src_all = ei[1].astype(np.int64)
    src_rank = src_all // NPC
    src_lr = src_all % NPC          # local row on owning core
    out = {}
    for layer, rel in ((0, REL0), (1, REL1)):
        sel = et == rel
        dst = dst_all[sel]
        rank = src_rank[sel]
        lr = src_lr[sel]
        groups = [[[None] * NT for _ in range(P)] for _ in range(2)]
        for c in range(P):
            m = (dst >= c * NPC) & (dst < (c + 1) * NPC)
            d_loc = (dst[m] - c * NPC).astype(np.int64)
            rk = rank[m]
            lrm = lr[m]
            for half in (0, 1):
                hm = (lrm < HA) if half == 0 else (lrm >= HA)
                sr = (rk[hm] * HA + lrm[hm] if half == 0
                      else rk[hm] * HB + (lrm[hm] - HA))
                dl = d_loc[hm]
                w = dl // 128
                order = np.argsort(w, kind="stable")
                sr, dl, w = sr[order], dl[order], w[order]
                idx = np.searchsorted(w, np.arange(NT + 1))
                for wi in range(NT):
                    groups[half][c][wi] = (sr[idx[wi]:idx[wi + 1]],
                                           dl[idx[wi]:idx[wi + 1]])
        for half in (0, 1):
            caps = [max(len(groups[half][c][w][0]) for c in range(P))
                    for w in range(NT)]
            L = sum(caps)
            Lpad = ((L + CHUNK - 1) // CHUNK) * CHUNK
            caps[-1] += Lpad - L
            bounds = np.concatenate([[0], np.cumsum(caps)])
            lists = []
            for c in range(P):
                srel = np.zeros(Lpad, np.int64)
                dloc = np.full(Lpad, -1, np.int64)
                for w in range(NT):
                    sr, dl = groups[half][c][w]
                    b = bounds[w]
                    srel[b:b + len(sr)] = sr
                    dloc[b:b + len(dl)] = dl
                lists.append((srel, dloc))
            out[(layer, half)] = (Lpad, bounds, lists)
    return out


def _prep_inputs(inputs):
    import ml_dtypes
    f = lambda k: np.asarray(inputs[k], dtype=np.float32)
    bf = lambda a: np.ascontiguousarray(a).astype(ml_dtypes.bfloat16)
    x = f("x")
    edges = _prep_edges(inputs["edge_index"], inputs["edge_type"])

    w1, b1 = f("mlp_w1"), f("mlp_b1")
    w2, b2 = f("mlp_w2"), f("mlp_b2")
    w3, b3 = f("mlp_w3"), f("mlp_b3")
    w01_0 = f("w0_0") + f("w1_0")
    ball0 = f("b0_0") + f("b1_0") + f("bl_0")
    w01_1 = f("w0_1") + f("w1_1")
    ball1 = f("b0_1") + f("b1_1") + f("bl_1")
    wl0, wl1 = f("wl_0"), f("wl_1")
    fc1s = f("fc1_w")[:D] + f("fc1_w")[D:]
    fc1b = f("fc1_b")
    fc2w, fc2b = f("fc2_w"), f("fc2_b")
    wcat0 = np.concatenate([wl0, w01_0], axis=1)   # [256, 256] -> [m0 | d0]
    wcat1 = np.concatenate([wl1, w01_1], axis=1)   # [128, 256] -> [m1 | d1]

    shared = {
        "w1": bf(w1), "w2": bf(w2), "w3": bf(w3),
        "b1": b1.reshape(D, 1), "b2": b2.reshape(D, 1),
        "b3a": b3[:D].reshape(D, 1), "b3b": b3[D:].reshape(D, 1),
        "wl0a": bf(wl0[:D]),
        "wl0b": bf(wl0[D:]),
        "w01a": bf(w01_0[:D]),
        "w01b": bf(w01_0[D:]),
        "wl1": bf(wl1), "w011": bf(w01_1),
        "ball0": ball0.reshape(D, 1), "ball1": ball1.reshape(D, 1),
        "fc1s": bf(fc1s), "fc1b": fc1b.reshape(D, 1),
        "fc2w": bf(fc2w),
        "fc2brep": np.tile(fc2b.reshape(1, NCLS), (128, 12)).astype(np.float32),
    }
    meta = {k: (v[0], v[1]) for k, v in edges.items()}
    # enumerate one-hot ops (w, half, tile) exactly as _build does
    sops = {}
    for layer in (0, 1):
        ops = []
        for w in range(NT):
            for half in (0, 1):
                Lpad, bounds, lists = edges[(layer, half)]
                t0 = bounds[w] // 128
                t1 = (bounds[w + 1] - 1) // 128
                for t in range(t0, t1 + 1):
                    ops.append((w, half, t))
        sops[layer] = ops

    in_maps = []
    for c in range(P):
        m = dict(shared)
        xt = np.zeros((D, NPP), np.float32)
        xt[:, :NPC] = x[c * NPC:(c + 1) * NPC].T
        m["xt"] = bf(xt)
        for (layer, half), (Lpad, bounds, lists) in edges.items():
            srel, dloc = lists[c]
            m[f"gs{layer}{half}"] = _wrap_idx(srel.astype(np.int16))
        import ml_dtypes
        for layer in (0, 1):
            ops = sops[layer]
            sall = np.zeros((128, len(ops) * 128), ml_dtypes.float8_e4m3)
            for i, (w, half, t) in enumerate(ops):
                dloc = edges[(layer, half)][2][c][1][t * 128:(t + 1) * 128]
                rel = dloc - 128 * w
                valid = (rel >= 0) & (rel < 128)
                e = np.nonzero(valid)[0]
                sall[e, i * 128 + rel[valid]] = 1.0
            m[f"sall{layer}"] = sall
        in_maps.append(m)
    return in_maps, meta


def _build(meta):
    nc = bacc.Bacc(None, target_bir_lowering=False, num_swdge_queues=4)

    def din(name, shape, dtype=F32):
        return nc.dram_tensor(name, list(shape), dtype, kind="ExternalInput")

    BF_W = {"w1", "w2", "w3", "wl0a", "wl0b", "w01a", "w01b", "wl1", "w011",
            "fc1s", "fc2w"}
    xt_d = din("xt", (D, NPP), BF16)
    wd = {}
    for name, shape in [
        ("w1", (D, D)), ("w2", (D, D)), ("w3", (D, H2)),
        ("b1", (D, 1)), ("b2", (D, 1)), ("b3a", (D, 1)), ("b3b", (D, 1)),
        ("wl0a", (D, D)), ("wl0b", (D, D)), ("w01a", (D, D)), ("w01b", (D, D)),
        ("wl1", (D, D)), ("w011", (D, D)),
        ("ball0", (D, 1)), ("ball1", (D, 1)),
        ("fc1s", (D, D)), ("fc1b", (D, 1)),
        ("fc2w", (D, NCLS)), ("fc2brep", (128, 12 * NCLS)),
    ]:
        wd[name] = din(name, shape, BF16 if name in BF_W else F32)
    idx_d = {}
    for (layer, half), (Lpad, bounds) in meta.items():
        idx_d[(layer, half, "s")] = din(f"gs{layer}{half}", (128, Lpad // 16), I16)
    nops = {}
    for layer in (0, 1):
        ops = []
        for w in range(NT):
            for half in (0, 1):
                Lpad, bounds = meta[(layer, half)]
                t0 = bounds[w] // 128
                t1 = (bounds[w + 1] - 1) // 128
                for t in range(t0, t1 + 1):
                    ops.append((w, half, t))
        nops[layer] = ops
        idx_d[(layer, "sall")] = din(f"sall{layer}", (128, len(ops) * 128), F8E4)

    m_own = [(nc.dram_tensor(f"m{i}_own_a", [HA, D], BF16),
              nc.dram_tensor(f"m{i}_own_b", [HB, D], BF16)) for i in range(2)]
    m_full = [(nc.dram_tensor(f"m{i}_full_a", [P * HA, D], BF16,
                              addr_space="Shared"),
               nc.dram_tensor(f"m{i}_full_b", [P * HB, D], BF16,
                              addr_space="Shared")) for i in range(2)]

    def m_own_dst(i, t):
        """DRAM row-slice of m{i}_own_{a,b} for local node tile t."""
        if t < NTA:
            return m_own[i][0][t * 128:(t + 1) * 128, :]
        return m_own[i][1][(t - NTA) * 128:(t - NTA + 1) * 128, :]
    # node-major tiles dumped partition-major: y[p, t*NCLS+c] = node t*128+p
    y_d = nc.dram_tensor("y", [128, NT * NCLS], F32, kind="ExternalOutput")

    AF = mybir.ActivationFunctionType
    ALU = mybir.AluOpType
    NCH = 13

    def chunks512():
        for i in range(NCH):
            lo = i * 512
            yield lo, min(512, NPP - lo)

    with TileContext(nc) as tc:
        with tc.tile_pool(name="const", bufs=1) as cpool:
            W = {}
            for name, t in wd.items():
                W[name] = cpool.tile(list(t.shape), t.dtype,
                                     tag=name, name=f"W_{name}")
                nc.sync.dma_start(out=W[name][:], in_=t[:])

            def body(rep):
              from contextlib import ExitStack
              with tc.tile_pool(name=f"persist{rep}", bufs=1) as pp:
                dterm = pp.tile([128, NPP], F32, name="dterm")     # node-major
                out_fm = pp.tile([128, NPP], BF16, name="out_fm")  # feature-major

                def load_graph_inputs(layer, gip, npieces=8):
                    """si/sall loads on the ACT HWDGE ring (overlaps other DMA).
                    sall is loaded in pieces so early consumers unblock early."""
                    pre = {}
                    for half in (0, 1):
                        Lpad, bounds = meta[(layer, half)]
                        si = gip.tile([128, Lpad // 16], I16,
                                      name=f"si{layer}_{half}", tag=f"si{half}")
                        nc.scalar.dma_start(out=si[:],
                                            in_=idx_d[(layer, half, "s")][:])
                        pre[half] = si
                    pre["sall"] = load_sall(layer, gip, npieces)
                    return pre

                def load_sall(layer, gip, npieces=8):
                    nop = len(nops[layer])
                    sall = gip.tile([128, nop * 128], F8E4, name=f"sall{layer}",
                                    tag="sall")
                    step = (nop + npieces - 1) // npieces * 128
                    for lo in range(0, nop * 128, step):
                        hi = min(lo + step, nop * 128)
                        nc.scalar.dma_start(out=sall[:, lo:hi],
                                            in_=idx_d[(layer, "sall")][:, lo:hi])
                    return sall

                # layer-0 graph inputs stream in during the MLP phase;
                # layer-1 si too (small). layer-1 sall streams during md1/AG1.
                stk1si = ExitStack()
                gip1si = stk1si.enter_context(
                    tc.tile_pool(name=f"gi{rep}_1si", bufs=1))
                pre1 = {}
                for half in (0, 1):
                    Lpad, bounds = meta[(1, half)]
                    si = gip1si.tile([128, Lpad // 16], I16,
                                     name=f"si1_{half}", tag=f"si1_{half}")
                    nc.scalar.dma_start(out=si[:], in_=idx_d[(1, half, "s")][:])
                    pre1[half] = si
                stk0 = ExitStack()
                gip0 = stk0.enter_context(tc.tile_pool(name=f"gi{rep}_0", bufs=1))
                pre0 = load_graph_inputs(0, gip0)

                # ---------------- Phase 1: MLP ----------------
                with (
                    tc.tile_pool(name=f"mlp{rep}", bufs=1) as mp,
                    tc.tile_pool(name=f"mlpc{rep}", bufs=3) as mpc,
                ):
                    xt = mp.tile([D, NPP], BF16, name="xt_s")
                    nc.sync.dma_start(out=xt[:], in_=xt_d[:])
                    h3 = [mp.tile([D, NPP], BF16, name=f"h3_{j}") for j in range(2)]
                    with tc.tile_pool(name=f"psA{rep}", bufs=2, space="PSUM") as psA:
                        for lo, w in chunks512():
                            ps1 = psA.tile([D, 512], F32, tag="ps1", name="ps1")
                            nc.tensor.matmul(ps1[:, :w], W["w1"][:], xt[:, lo:lo + w])
                            h1 = mpc.tile([D, 512], BF16, tag="h1", name="h1")
                            nc.scalar.activation(h1[:, :w], ps1[:, :w], AF.Relu,
                                                 bias=W["b1"][:])
                            ps2 = psA.tile([D, 512], F32, tag="ps2", name="ps2")
                            nc.tensor.matmul(ps2[:, :w], W["w2"][:], h1[:, :w])
                            h2t = mpc.tile([D, 512], BF16, tag="h2", name="h2")
                            nc.scalar.activation(h2t[:, :w], ps2[:, :w], AF.Relu,
                                                 bias=W["b2"][:])
                            for j in range(2):
                                ps3 = psA.tile([D, 512], F32, tag="ps3", name="ps3")
                                nc.tensor.matmul(
                                    ps3[:, :w], W["w3"][:, j * D:(j + 1) * D],
                                    h2t[:, :w]
                                )
                                nc.vector.tensor_scalar_add(
                                    h3[j][:, lo:lo + w], ps3[:, :w],
                                    W["b3a"][:] if j == 0 else W["b3b"][:],
                                )
                    # m0 node-major (for gather rows) + d0 feature-major into dterm
                    with (
                        tc.tile_pool(name=f"md0{rep}", bufs=4) as md0p,
                        tc.tile_pool(name=f"psB{rep}", bufs=3, space="PSUM") as psB,
                    ):
                      if PHASES >= 2:
                        for t0g in range(0, NT, 4):
                            tg = list(range(t0g, min(t0g + 4, NT)))
                            ps = psB.tile([128, 512], F32, tag="m0ps", name="m0ps")
                            for j, t in enumerate(tg):
                                lo = t * 128
                                nc.tensor.matmul(ps[:, j * 128:(j + 1) * 128],
                                                 h3[0][:, lo:lo + 128], W["wl0a"][:],
                                                 start=True, stop=False,
                                                 skip_group_check=True)
                                nc.tensor.matmul(ps[:, j * 128:(j + 1) * 128],
                                                 h3[1][:, lo:lo + 128], W["wl0b"][:],
                                                 start=False, stop=True,
                                                 skip_group_check=True)
                            gw = len(tg) * 128
                            m0t = md0p.tile([128, 512], BF16, tag="m0t", name="m0t")
                            nc.vector.tensor_copy(m0t[:, :gw], ps[:, :gw])
                            for j, t in enumerate(tg):
                                nc.sync.dma_start(out=m_own_dst(0, t),
                                                  in_=m0t[:, j * 128:(j + 1) * 128])
                        for lo, w in chunks512():
                            psd = psB.tile([D, 512], F32, tag="d0ps", name="d0ps")
                            nc.tensor.matmul(psd[:, :w], W["w01a"][:], h3[0][:, lo:lo + w],
                                             start=True, stop=False)
                            nc.tensor.matmul(psd[:, :w], W["w01b"][:], h3[1][:, lo:lo + w],
                                             start=False, stop=True)
                            nc.scalar.activation(dterm[:, lo:lo + w], psd[:, :w],
                                                 AF.Identity, bias=W["ball0"][:])
                if PHASES < 2:
                    stk0.close()
                    stk1si.close()
                    return

                def allgather(i):
                    if SKIP_AG:
                        return
                    for h in (0, 1):
                        nc.gpsimd.collective_compute(
                            "AllGather", mybir.AluOpType.bypass,
                            ins=[m_own[i][h][:]], outs=[m_full[i][h][:]],
                            replica_groups=[list(range(P))],
                        )

                def graph_layer(layer, pre=None):
                    """PE one-hot segment sum + relu epilogue -> out_fm."""
                    if SKIP_GRAPH:
                        with tc.tile_pool(name=f"sg{rep}_{layer}", bufs=4) as sgp, \
                             tc.tile_pool(name=f"sgp{rep}_{layer}", bufs=3, space="PSUM") as sgps:
                            for w in range(NT):
                                blk = slice(w * 128, (w + 1) * 128)
                                srel = sgp.tile([128, 128], F32, tag="srel", name="srel")
                                nc.scalar.activation(srel[:], dterm[:, blk], AF.Relu)
                                pt = sgps.tile([128, 128], F32, tag="pt", name="pt")
                                nc.tensor.matmul(pt[:], srel[:], W["ident"][:], is_transpose=True)
                                nc.vector.tensor_copy(out_fm[:, blk], pt[:])
                        return
                    with ExitStack() as lstk:
                        gpa = lstk.enter_context(
                            tc.tile_pool(name=f"gsa{rep}_{layer}", bufs=10))
                        gpb = lstk.enter_context(
                            tc.tile_pool(name=f"gsb{rep}_{layer}", bufs=10))
                        psw = lstk.enter_context(
                            tc.tile_pool(name=f"ps{rep}_{layer}", bufs=6,
                                         space="PSUM"))
                        ep = lstk.enter_context(
                            tc.tile_pool(name=f"ep{rep}_{layer}", bufs=4))
                        if pre is None:
                            gip = lstk.enter_context(
                                tc.tile_pool(name=f"gi{rep}_{layer}", bufs=1))
                            pre = load_graph_inputs(layer, gip)
                        halves = {}
                        for half in (0, 1):
                            Lpad, bounds = meta[(layer, half)]
                            halves[half] = (Lpad, bounds, pre[half], None, {})
                        sall = pre["sall"]
                        opctr = [0]

                        src_view = [m_full[layer][0][:], m_full[layer][1][:]]
                        gpool = {0: gpa, 1: gpb}
                        qctr = [0]

                        def get_chunk(half, c):
                            Lpad, bounds, si, dl, bufs = halves[half]
                            gp = gpool[half]
                            if SKIP_GATHER:
                                if "z" not in bufs:
                                    g = CHUNK // 128
                                    bufs["z"] = gp.tile([128, g, D], BF16, tag="gbuf", name=f"gbz{half}")
                                    nc.vector.memset(bufs["z"][:], 0)
                                return bufs["z"]
                            if c not in bufs:
                                g = CHUNK // 128
                                buf = gp.tile([128, g, D], BF16, tag="gbuf",
                                              name=f"gb{half}_{c}")
                                nc.gpsimd.dma_gather(
                                    buf[:], src_view[half],
                                    si[:, c * CHUNK // 16:(c + 1) * CHUNK // 16],
                                    CHUNK, CHUNK, D, queue_num=qctr[0] % 4,
                                )
                                qctr[0] += 1
                                bufs[c] = buf
                            return bufs[c]

                        # issue all gathers upfront (half A first: its
                        # AllGather completes first, half-A gathers overlap
                        # half B's transfer)
                        if not SKIP_GATHER and PREFETCH:
                            for half in (0, 1):
                                Lpad = halves[half][0]
                                for c in range(Lpad // CHUNK):
                                    get_chunk(half, c)

                        for w0 in range(0, NT, 4):
                            ws = list(range(w0, min(w0 + 4, NT)))
                            pw = psw.tile([128, 512], F32, tag="pw", name="pw")
                            for w in ws:
                                off = (w - w0) * 128
                                ops = []  # (half, tile_idx)
                                for half in (0, 1):
                                    Lpad, bounds, si, dl, bufs = halves[half]
                                    t0 = bounds[w] // 128
                                    t1 = (bounds[w + 1] - 1) // 128
                                    for t in range(t0, t1 + 1):
                                        ops.append((half, t))
                                for i, (half, t) in enumerate(ops):
                                    buf = get_chunk(half, t * 128 // CHUNK)
                                    slot = (t * 128 % CHUNK) // 128
                                    oc = opctr[0]
                                    opctr[0] += 1
                                    nc.tensor.matmul(
                                        pw[:, off:off + 128],
                                        buf[:, slot, :],
                                        sall[:, oc * 128:(oc + 1) * 128],
                                        start=(i == 0), stop=(i == len(ops) - 1),
                                        skip_group_check=True,
                                    )
                            gw = len(ws) * 128
                            blk = slice(w0 * 128, w0 * 128 + gw)
                            sadd = ep.tile([128, 512], F32, tag="sadd", name="sadd")
                            nc.vector.tensor_add(sadd[:, :gw], pw[:, :gw], dterm[:, blk])
                            nc.scalar.activation(out_fm[:, blk], sadd[:, :gw], AF.Relu)

                # ---------------- Layer 0 ----------------
                allgather(0)
                if PHASES < 3:
                    stk0.close()
                    stk1si.close()
                    return
                graph_layer(0, pre0)
                stk0.close()
                if PHASES < 4:
                    stk1si.close()
                    return
                # layer-1 sall streams in during md1 + AG1
                stk1 = ExitStack()
                gip1 = stk1.enter_context(tc.tile_pool(name=f"gi{rep}_1sall", bufs=1))
                pre1["sall"] = load_sall(1, gip1)
                # m1|d1 from out_fm; overwrite dterm with layer-1 dense term
                with (
                    tc.tile_pool(name=f"md1{rep}", bufs=4) as md1p,
                    tc.tile_pool(name=f"psC{rep}", bufs=3, space="PSUM") as psC,
                ):
                    for t0g in range(0, NT, 4):
                        tg = list(range(t0g, min(t0g + 4, NT)))
                        ps = psC.tile([128, 512], F32, tag="m1ps", name="m1ps")
                        for j, t in enumerate(tg):
                            lo = t * 128
                            nc.tensor.matmul(ps[:, j * 128:(j + 1) * 128],
                                             out_fm[:, lo:lo + 128], W["wl1"][:],
                                             skip_group_check=True)
                        gw = len(tg) * 128
                        m1t = md1p.tile([128, 512], BF16, tag="m1t", name="m1t")
                        nc.vector.tensor_copy(m1t[:, :gw], ps[:, :gw])
                        for j, t in enumerate(tg):
                            nc.sync.dma_start(out=m_own_dst(1, t),
                                              in_=m1t[:, j * 128:(j + 1) * 128])
                    for lo, w in chunks512():
                        psd = psC.tile([D, 512], F32, tag="d1ps", name="d1ps")
                        nc.tensor.matmul(psd[:, :w], W["w011"][:], out_fm[:, lo:lo + w])
                        nc.scalar.activation(dterm[:, lo:lo + w], psd[:, :w],
                                             AF.Identity, bias=W["ball1"][:])

                # ---------------- Layer 1 ----------------
                allgather(1)
                if PHASES < 5:
                    stk1.close()
                    stk1si.close()
                    return
                graph_layer(1, pre1)
                if PHASES < 6:
                    stk1.close()
                    stk1si.close()
                    return

                # ---------------- Classifier + log_softmax ----------------
                # fc2/softmax in node-major: nodes on partitions, per-node
                # log-sum-exp via free-dim segmented reduce.
                with (
                    tc.tile_pool(name=f"fc{rep}", bufs=4) as fcp,
                    tc.tile_pool(name=f"fcb{rep}", bufs=1) as fcbp,
                    tc.tile_pool(name=f"psD{rep}", bufs=2, space="PSUM") as psD,
                ):
                    tfm = fcbp.tile([128, NPP], BF16, name="tfm")
                    for lo, w in chunks512():
                        ps = psD.tile([D, 512], F32, tag="fc1ps", name="fc1ps")
                        nc.tensor.matmul(ps[:, :w], W["fc1s"][:], out_fm[:, lo:lo + w])
                        nc.scalar.activation(tfm[:, lo:lo + w], ps[:, :w], AF.Relu,
                                             bias=W["fc1b"][:])
                    for g0 in range(0, NT, 12):
                        tg = list(range(g0, min(g0 + 12, NT)))
                        ng = len(tg)
                        gw = ng * NCLS
                        ps = psD.tile([128, 12 * NCLS], F32, tag="fc2ps", name="fc2ps")
                        for j, t in enumerate(tg):
                            nc.tensor.matmul(ps[:, j * NCLS:(j + 1) * NCLS],
                                             tfm[:, t * 128:(t + 1) * 128],
                                             W["fc2w"][:], skip_group_check=True)
                        lg = fcp.tile([128, 12 * NCLS], F32, tag="lg", name="lg")
                        nc.vector.tensor_add(lg[:, :gw], ps[:, :gw],
                                             W["fc2brep"][:, :gw])
                        ex = fcp.tile([128, 12 * NCLS], BF16, tag="ex", name="ex")
                        nc.scalar.activation(ex[:, :gw], lg[:, :gw], AF.Exp)
                        s = fcp.tile([128, 12], F32, tag="s", name="s")
                        nc.vector.reduce_sum(
                            s[:, :ng],
                            ex[:, :gw].rearrange("p (n c) -> p n c", c=NCLS),
                            axis=mybir.AxisListType.X)
                        lns = fcp.tile([128, 12], F32, tag="lns", name="lns")
                        nc.scalar.activation(lns[:, :ng], s[:, :ng], AF.Ln)
                        yt = fcp.tile([128, 12 * NCLS], F32, tag="yt", name="yt")
                        nc.vector.tensor_sub(
                            yt[:, :gw].rearrange("p (n c) -> p n c", c=NCLS),
                            lg[:, :gw].rearrange("p (n c) -> p n c", c=NCLS),
                            lns[:, :ng].rearrange("p (n o) -> p n o", o=1)
                               .to_broadcast([128, ng, NCLS]))
                        nc.sync.dma_start(
                            out=y_d[:, g0 * NCLS:g0 * NCLS + gw],
                            in_=yt[:, :gw])
                stk1.close()
                stk1si.close()
            for rep in range(REPEAT):
                body(rep)
    nc.compile()
    return nc


def kernel(**inputs):
    global LAST_EXEC_NS, LAST_RESULTS
    h = hashlib.md5()
    for k in sorted(inputs):
        h.update(np.ascontiguousarray(np.asarray(inputs[k])).tobytes())
    key = f"{REPEAT}{SKIP_AG}{SKIP_GATHER}{SKIP_GRAPH}{PREFETCH}{PHASES}" + h.hexdigest()
    if key not in _CACHE:
        in_maps, meta = _prep_inputs(inputs)
        nc = _build({k: (v[0], tuple(v[1])) for k, v in meta.items()})
        _CACHE[key] = (nc, in_maps)
    nc, in_maps = _CACHE[key]
    res = run_bass_kernel_spmd(nc, in_maps, list(range(P)), trace=TRACE, **TRACE_KW)
    LAST_EXEC_NS = res.exec_time_ns
    LAST_RESULTS = res
    outs = res.results
    y = np.concatenate(
        [outs[c]["y"].reshape(128, NT, NCLS).transpose(1, 0, 2)
         .reshape(NPP, NCLS)[:NPC] for c in range(P)],
        axis=0)
    return np.ascontiguousarray(y, dtype=np.float32)



# revision 57
# speedup vs baseline: 132.7939x; 1.0157x over previous
"""MetaPathGNN Trainium2 kernel: 8-core SPMD, node-sharded. ~427us/iter.

Host (untimed): edge filtering/sorting/partitioning, weight folding, layout prep.
Device: bf16 feature-major MLP, AllGather (2 row-halves; the half split doubles
as the int16 gather-index split) of bf16 projected messages, 4-queue SWDGE
dma_gather of per-edge source rows, PE one-hot (fp8e4) matmul segment-sum with
PSUM accumulation per 128-dst window, node-major fc2 + log_softmax via
free-dim segmented reduce. Layer-0 sall/si preloaded during the MLP; layer-1
sall streamed in pieces during md1/AG1.

Measured bottleneck structure (per rep): AllGathers ~130us + gathers ~125us
(4-SWDGE-queue limit, ~2.2ns/row) + compute ~160us; AG/gather overlap attempts
consistently LOST time (shared SDMA/HBM resources) - serial at full rate beats
overlapped at half rate here.
"""

import hashlib
import sys

import numpy as np

sys.path.insert(0, "/opt/trn_rl_repo")

import concourse.bass as bass
import concourse.bacc as bacc
import concourse.mybir as mybir
from concourse.bass_utils import run_bass_kernel_spmd
from concourse.tile import TileContext

N = 50000
P = 8
NPC = 6250          # nodes per core
NPP = 6272          # padded: 49 * 128
NT = NPP // 128     # 49 node tiles / dst windows per core
D = 128
H2 = 256
NCLS = 40
REL0, REL1 = 2, 3
# messages AllGathered in two row-halves (also keeps gather idx in int16):
# half A = local rows [0, HA), half B = [HA, NPP)
HA = 3200           # 25 tiles
HB = NPP - HA       # 3072: 24 tiles
NTA = HA // 128
CHUNK = 1024        # gather chunk (descriptor ring tops out < 2048)

F32 = mybir.dt.float32
BF16 = mybir.dt.bfloat16
F8E4 = mybir.dt.float8e4
I16 = mybir.dt.int16

import os
REPEAT = int(os.environ.get("KREPEAT", "1"))
SKIP_AG = os.environ.get("SKIP_AG") == "1"
SKIP_GATHER = os.environ.get("SKIP_GATHER") == "1"
SKIP_GRAPH = os.environ.get("SKIP_GRAPH") == "1"
PREFETCH = False
PHASES = 9  # truncate body after phase k (1=MLP,2=+md0+AG0,3=+graph0,4=+md1+AG1,5=+graph1,9=all)
_CACHE = {}
LAST_EXEC_NS = None
LAST_RESULTS = None
TRACE = False
TRACE_KW = {}


def _wrap_idx(a):
    """[L] int16 -> [128, L/16] in (s p) wrapped layout, replicated for 8 q7 cores."""
    sb = a.reshape(-1, 16).T.copy()
    return np.tile(sb, (8, 1))


def _prep_edges(edge_index, edge_type):
    """Per (layer, half): uniform-cap window-sorted edge streams.

    Stream = concat over dst-window w of that window's edges, padded per window
    to cap_w (max count over cores) with (src=0, dstloc=-1) null edges; total
    padded to a CHUNK multiple (tail assigned to the last window).
    Returns dict[(layer, half)] -> (L, bounds, per_core list of (srel, dstloc)).
    bounds[w] = start position of window w in the stream (static, shared).
    """
    ei = np.asarray(edge_index)
    et = np.asarray(edge_type)
    dst_all = ei[0].astype(np.int64)
    src_all = ei[1].astype(np.int64)
    src_rank = src_all // NPC
    src_lr = src_all % NPC          # local row on owning core
    out = {}
    for layer, rel in ((0, REL0), (1, REL1)):
        sel = et == rel
        dst = dst_all[sel]
        rank = src_rank[sel]
        lr = src_lr[sel]
        groups = [[[None] * NT for _ in range(P)] for _ in range(2)]
        for c in range(P):
            m = (dst >= c * NPC) & (dst < (c + 1) * NPC)
            d_loc = (dst[m] - c * NPC).astype(np.int64)
            rk = rank[m]
            lrm = lr[m]
            for half in (0, 1):
                hm = (lrm < HA) if half == 0 else (lrm >= HA)
                sr = (rk[hm] * HA + lrm[hm] if half == 0
                      else rk[hm] * HB + (lrm[hm] - HA))
                dl = d_loc[hm]
                w = dl // 128
                order = np.argsort(w, kind="stable")
                sr, dl, w = sr[order], dl[order], w[order]
                idx = np.searchsorted(w, np.arange(NT + 1))
                for wi in range(NT):
                    groups[half][c][wi] = (sr[idx[wi]:idx[wi + 1]],
                                           dl[idx[wi]:idx[wi + 1]])
        for half in (0, 1):
            caps = [max(len(groups[half][c][w][0]) for c in range(P))
                    for w in range(NT)]
            L = sum(caps)
            Lpad = ((L + CHUNK - 1) // CHUNK) * CHUNK
            caps[-1] += Lpad - L
            bounds = np.concatenate([[0], np.cumsum(caps)])
            lists = []
            for c in range(P):
                srel = np.zeros(Lpad, np.int64)
                dloc = np.full(Lpad, -1, np.int64)
                for w in range(NT):
                    sr, dl = groups[half][c][w]
                    b = bounds[w]
                    srel[b:b + len(sr)] = sr
                    dloc[b:b + len(dl)] = dl
                lists.append((srel, dloc))
            out[(layer, half)] = (Lpad, bounds, lists)
    return out


def _prep_inputs(inputs):
    import ml_dtypes
    f = lambda k: np.asarray(inputs[k], dtype=np.float32)
    bf = lambda a: np.ascontiguousarray(a).astype(ml_dtypes.bfloat16)
    x = f("x")
    edges = _prep_edges(inputs["edge_index"], inputs["edge_type"])

    w1, b1 = f("mlp_w1"), f("mlp_b1")
    w2, b2 = f("mlp_w2"), f("mlp_b2")
    w3, b3 = f("mlp_w3"), f("mlp_b3")
    w01_0 = f("w0_0") + f("w1_0")
    ball0 = f("b0_0") + f("b1_0") + f("bl_0")
    w01_1 = f("w0_1") + f("w1_1")
    ball1 = f("b0_1") + f("b1_1") + f("bl_1")
    wl0, wl1 = f("wl_0"), f("wl_1")
    fc1s = f("fc1_w")[:D] + f("fc1_w")[D:]
    fc1b = f("fc1_b")
    fc2w, fc2b = f("fc2_w"), f("fc2_b")
    wcat0 = np.concatenate([wl0, w01_0], axis=1)   # [256, 256] -> [m0 | d0]
    wcat1 = np.concatenate([wl1, w01_1], axis=1)   # [128, 256] -> [m1 | d1]

    shared = {
        "w1": bf(w1), "w2": bf(w2), "w3": bf(w3),
        "b1": b1.reshape(D, 1), "b2": b2.reshape(D, 1),
        "b3a": b3[:D].reshape(D, 1), "b3b": b3[D:].reshape(D, 1),
        "wl0a": bf(wl0[:D]),
        "wl0b": bf(wl0[D:]),
        "w01a": bf(w01_0[:D]),
        "w01b": bf(w01_0[D:]),
        "wl1": bf(wl1), "w011": bf(w01_1),
        "ball0": ball0.reshape(D, 1), "ball1": ball1.reshape(D, 1),
        "fc1s": bf(fc1s), "fc1b": fc1b.reshape(D, 1),
        "fc2w": bf(fc2w),
        "fc2brep": np.tile(fc2b.reshape(1, NCLS), (128, 12)).astype(np.float32),
    }
    meta = {k: (v[0], v[1]) for k, v in edges.items()}
    # enumerate one-hot ops (w, half, tile) exactly as _build does
    sops = {}
    for layer in (0, 1):
        ops = []
        for w in range(NT):
            for half in (0, 1):
                Lpad, bounds, lists = edges[(layer, half)]
                t0 = bounds[w] // 128
                t1 = (bounds[w + 1] - 1) // 128
                for t in range(t0, t1 + 1):
                    ops.append((w, half, t))
        sops[layer] = ops

    in_maps = []
    for c in range(P):
        m = dict(shared)
        xt = np.zeros((D, NPP), np.float32)
        xt[:, :NPC] = x[c * NPC:(c + 1) * NPC].T
        m["xt"] = bf(xt)
        for (layer, half), (Lpad, bounds, lists) in edges.items():
            srel, dloc = lists[c]
            m[f"gs{layer}{half}"] = _wrap_idx(srel.astype(np.int16))
        import ml_dtypes
        for layer in (0, 1):
            ops = sops[layer]
            sall = np.zeros((128, len(ops) * 128), ml_dtypes.float8_e4m3)
            for i, (w, half, t) in enumerate(ops):
                dloc = edges[(layer, half)][2][c][1][t * 128:(t + 1) * 128]
                rel = dloc - 128 * w
                valid = (rel >= 0) & (rel < 128)
                e = np.nonzero(valid)[0]
                sall[e, i * 128 + rel[valid]] = 1.0
            m[f"sall{layer}"] = sall
        in_maps.append(m)
    return in_maps, meta


def _build(meta):
    nc = bacc.Bacc(None, target_bir_lowering=False, num_swdge_queues=4)

    def din(name, shape, dtype=F32):
        return nc.dram_tensor(name, list(shape), dtype, kind="ExternalInput")

    BF_W = {"w1", "w2", "w3", "wl0a", "wl0b", "w01a", "w01b", "wl1", "w011",
            "fc1s", "fc2w"}
    xt_d = din("xt", (D, NPP), BF16)
    wd = {}
    for name, shape in [
        ("w1", (D, D)), ("w2", (D, D)), ("w3", (D, H2)),
        ("b1", (D, 1)), ("b2", (D, 1)), ("b3a", (D, 1)), ("b3b", (D, 1)),
        ("wl0a", (D, D)), ("wl0b", (D, D)), ("w01a", (D, D)), ("w01b", (D, D)),
        ("wl1", (D, D)), ("w011", (D, D)),
        ("ball0", (D, 1)), ("ball1", (D, 1)),
        ("fc1s", (D, D)), ("fc1b", (D, 1)),
        ("fc2w", (D, NCLS)), ("fc2brep", (128, 12 * NCLS)),
    ]:
        wd[name] = din(name, shape, BF16 if name in BF_W else F32)
    idx_d = {}
    for (layer, half), (Lpad, bounds) in meta.items():
        idx_d[(layer, half, "s")] = din(f"gs{layer}{half}", (128, Lpad // 16), I16)
    nops = {}
    for layer in (0, 1):
        ops = []
        for w in range(NT):
            for half in (0, 1):
                Lpad, bounds = meta[(layer, half)]
                t0 = bounds[w] // 128
                t1 = (bounds[w + 1] - 1) // 128
                for t in range(t0, t1 + 1):
                    ops.append((w, half, t))
        nops[layer] = ops
        idx_d[(layer, "sall")] = din(f"sall{layer}", (128, len(ops) * 128), F8E4)

    m_own = [(nc.dram_tensor(f"m{i}_own_a", [HA, D], BF16),
              nc.dram_tensor(f"m{i}_own_b", [HB, D], BF16)) for i in range(2)]
    m_full = [(nc.dram_tensor(f"m{i}_full_a", [P * HA, D], BF16,
                              addr_space="Shared"),
               nc.dram_tensor(f"m{i}_full_b", [P * HB, D], BF16,
                              addr_space="Shared")) for i in range(2)]

    def m_own_dst(i, t):
        """DRAM row-slice of m{i}_own_{a,b} for local node tile t."""
        if t < NTA:
            return m_own[i][0][t * 128:(t + 1) * 128, :]
        return m_own[i][1][(t - NTA) * 128:(t - NTA + 1) * 128, :]
    # node-major tiles dumped partition-major: y[p, t*NCLS+c] = node t*128+p
    y_d = nc.dram_tensor("y", [128, NT * NCLS], F32, kind="ExternalOutput")

    AF = mybir.ActivationFunctionType
    ALU = mybir.AluOpType
    NCH = 13

    def chunks512():
        for i in range(NCH):
            lo = i * 512
            yield lo, min(512, NPP - lo)

    with TileContext(nc) as tc:
        with tc.tile_pool(name="const", bufs=1) as cpool:
            W = {}
            for name, t in wd.items():
                W[name] = cpool.tile(list(t.shape), t.dtype,
                                     tag=name, name=f"W_{name}")
                nc.sync.dma_start(out=W[name][:], in_=t[:])

            def body(rep):
              from contextlib import ExitStack
              with tc.tile_pool(name=f"persist{rep}", bufs=1) as pp:
                dterm = pp.tile([128, NPP], F32, name="dterm")     # node-major
                out_fm = pp.tile([128, NPP], BF16, name="out_fm")  # feature-major

                def allgather(i):
                    if SKIP_AG:
                        return
                    for h in (0, 1):
                        nc.gpsimd.collective_compute(
                            "AllGather", mybir.AluOpType.bypass,
                            ins=[m_own[i][h][:]], outs=[m_full[i][h][:]],
                            replica_groups=[list(range(P))],
                        )

                def load_graph_inputs(layer, gip, npieces=8):
                    """si/sall loads on the ACT HWDGE ring (overlaps other DMA).
                    sall is loaded in pieces so early consumers unblock early."""
                    pre = {}
                    for half in (0, 1):
                        Lpad, bounds = meta[(layer, half)]
                        si = gip.tile([128, Lpad // 16], I16,
                                      name=f"si{layer}_{half}", tag=f"si{half}")
                        nc.scalar.dma_start(out=si[:],
                                            in_=idx_d[(layer, half, "s")][:])
                        pre[half] = si
                    pre["sall"] = load_sall(layer, gip, npieces)
                    return pre

                def load_sall(layer, gip, npieces=8):
                    nop = len(nops[layer])
                    sall = gip.tile([128, nop * 128], F8E4, name=f"sall{layer}",
                                    tag="sall")
                    step = (nop + npieces - 1) // npieces * 128
                    for lo in range(0, nop * 128, step):
                        hi = min(lo + step, nop * 128)
                        nc.scalar.dma_start(out=sall[:, lo:hi],
                                            in_=idx_d[(layer, "sall")][:, lo:hi])
                    return sall

                # layer-0 graph inputs stream in during the MLP phase;
                # layer-1 si too (small). layer-1 sall streams during md1/AG1.
                stk1si = ExitStack()
                gip1si = stk1si.enter_context(
                    tc.tile_pool(name=f"gi{rep}_1si", bufs=1))
                pre1 = {}
                for half in (0, 1):
                    Lpad, bounds = meta[(1, half)]
                    si = gip1si.tile([128, Lpad // 16], I16,
                                     name=f"si1_{half}", tag=f"si1_{half}")
                    nc.scalar.dma_start(out=si[:], in_=idx_d[(1, half, "s")][:])
                    pre1[half] = si
                stk0 = ExitStack()
                gip0 = stk0.enter_context(tc.tile_pool(name=f"gi{rep}_0", bufs=1))
                pre0 = load_graph_inputs(0, gip0)

                # ---------------- Phase 1: MLP ----------------
                with (
                    tc.tile_pool(name=f"mlp{rep}", bufs=1) as mp,
                    tc.tile_pool(name=f"mlpc{rep}", bufs=3) as mpc,
                ):
                    xt = mp.tile([D, NPP], BF16, name="xt_s")
                    nc.sync.dma_start(out=xt[:], in_=xt_d[:])
                    h3 = [mp.tile([D, NPP], BF16, name=f"h3_{j}") for j in range(2)]
                    with tc.tile_pool(name=f"psA{rep}", bufs=2, space="PSUM") as psA:
                        for lo, w in chunks512():
                            ps1 = psA.tile([D, 512], F32, tag="ps1", name="ps1")
                            nc.tensor.matmul(ps1[:, :w], W["w1"][:], xt[:, lo:lo + w])
                            h1 = mpc.tile([D, 512], BF16, tag="h1", name="h1")
                            nc.scalar.activation(h1[:, :w], ps1[:, :w], AF.Relu,
                                                 bias=W["b1"][:])
                            ps2 = psA.tile([D, 512], F32, tag="ps2", name="ps2")
                            nc.tensor.matmul(ps2[:, :w], W["w2"][:], h1[:, :w])
                            h2t = mpc.tile([D, 512], BF16, tag="h2", name="h2")
                            nc.scalar.activation(h2t[:, :w], ps2[:, :w], AF.Relu,
                                                 bias=W["b2"][:])
                            for j in range(2):
                                ps3 = psA.tile([D, 512], F32, tag="ps3", name="ps3")
                                nc.tensor.matmul(
                                    ps3[:, :w], W["w3"][:, j * D:(j + 1) * D],
                                    h2t[:, :w]
                                )
                                nc.vector.tensor_scalar_add(
                                    h3[j][:, lo:lo + w], ps3[:, :w],
                                    W["b3a"][:] if j == 0 else W["b3b"][:],
                                )
                    # m0 node-major (for gather rows) + d0 feature-major into dterm
                    with (
                        tc.tile_pool(name=f"md0{rep}", bufs=4) as md0p,
                        tc.tile_pool(name=f"psB{rep}", bufs=3, space="PSUM") as psB,
                    ):
                      if PHASES >= 2:
                        for t0g in range(0, NT, 4):
                            tg = list(range(t0g, min(t0g + 4, NT)))
                            ps = psB.tile([128, 512], F32, tag="m0ps", name="m0ps")
                            for j, t in enumerate(tg):
                                lo = t * 128
                                nc.tensor.matmul(ps[:, j * 128:(j + 1) * 128],
                                                 h3[0][:, lo:lo + 128], W["wl0a"][:],
                                                 start=True, stop=False,
                                                 skip_group_check=True)
                                nc.tensor.matmul(ps[:, j * 128:(j + 1) * 128],
                                                 h3[1][:, lo:lo + 128], W["wl0b"][:],
                                                 start=False, stop=True,
                                                 skip_group_check=True)
                            gw = len(tg) * 128
                            m0t = md0p.tile([128, 512], BF16, tag="m0t", name="m0t")
                            nc.vector.tensor_copy(m0t[:, :gw], ps[:, :gw])
                            for j, t in enumerate(tg):
                                nc.sync.dma_start(out=m_own_dst(0, t),
                                                  in_=m0t[:, j * 128:(j + 1) * 128])
                        allgather(0)
                        for lo, w in chunks512():
                            psd = psB.tile([D, 512], F32, tag="d0ps", name="d0ps")
                            nc.tensor.matmul(psd[:, :w], W["w01a"][:], h3[0][:, lo:lo + w],
                                             start=True, stop=False)
                            nc.tensor.matmul(psd[:, :w], W["w01b"][:], h3[1][:, lo:lo + w],
                                             start=False, stop=True)
                            nc.scalar.activation(dterm[:, lo:lo + w], psd[:, :w],
                                                 AF.Identity, bias=W["ball0"][:])
                if PHASES < 2:
                    stk0.close()
                    stk1si.close()
                    return

                def graph_layer(layer, pre=None):
                    """PE one-hot segment sum + relu epilogue -> out_fm."""
                    if SKIP_GRAPH:
                        with tc.tile_pool(name=f"sg{rep}_{layer}", bufs=4) as sgp, \
                             tc.tile_pool(name=f"sgp{rep}_{layer}", bufs=3, space="PSUM") as sgps:
                            for w in range(NT):
                                blk = slice(w * 128, (w + 1) * 128)
                                srel = sgp.tile([128, 128], F32, tag="srel", name="srel")
                                nc.scalar.activation(srel[:], dterm[:, blk], AF.Relu)
                                pt = sgps.tile([128, 128], F32, tag="pt", name="pt")
                                nc.tensor.matmul(pt[:], srel[:], W["ident"][:], is_transpose=True)
                                nc.vector.tensor_copy(out_fm[:, blk], pt[:])
                        return
                    with ExitStack() as lstk:
                        gpa = lstk.enter_context(
                            tc.tile_pool(name=f"gsa{rep}_{layer}", bufs=12))
                        gpb = lstk.enter_context(
                            tc.tile_pool(name=f"gsb{rep}_{layer}", bufs=12))
                        psw = lstk.enter_context(
                            tc.tile_pool(name=f"ps{rep}_{layer}", bufs=7,
                                         space="PSUM"))
                        ep = lstk.enter_context(
                            tc.tile_pool(name=f"ep{rep}_{layer}", bufs=4))
                        if pre is None:
                            gip = lstk.enter_context(
                                tc.tile_pool(name=f"gi{rep}_{layer}", bufs=1))
                            pre = load_graph_inputs(layer, gip)
                        halves = {}
                        for half in (0, 1):
                            Lpad, bounds = meta[(layer, half)]
                            halves[half] = (Lpad, bounds, pre[half], None, {})
                        sall = pre["sall"]
                        opctr = [0]

                        src_view = [m_full[layer][0][:], m_full[layer][1][:]]
                        gpool = {0: gpa, 1: gpb}
                        qctr = [0]

                        def get_chunk(half, c):
                            Lpad, bounds, si, dl, bufs = halves[half]
                            gp = gpool[half]
                            if SKIP_GATHER:
                                if "z" not in bufs:
                                    g = CHUNK // 128
                                    bufs["z"] = gp.tile([128, g, D], BF16, tag="gbuf", name=f"gbz{half}")
                                    nc.vector.memset(bufs["z"][:], 0)
                                return bufs["z"]
                            if c not in bufs:
                                g = CHUNK // 128
                                buf = gp.tile([128, g, D], BF16, tag="gbuf",
                                              name=f"gb{half}_{c}")
                                nc.gpsimd.dma_gather(
                                    buf[:], src_view[half],
                                    si[:, c * CHUNK // 16:(c + 1) * CHUNK // 16],
                                    CHUNK, CHUNK, D, queue_num=qctr[0] % 4,
                                )
                                qctr[0] += 1
                                bufs[c] = buf
                            return bufs[c]

                        # issue all gathers upfront (half A first: its
                        # AllGather completes first, half-A gathers overlap
                        # half B's transfer)
                        if not SKIP_GATHER and PREFETCH:
                            for half in (0, 1):
                                Lpad = halves[half][0]
                                for c in range(Lpad // CHUNK):
                                    get_chunk(half, c)

                        for w0 in range(0, NT, 4):
                            ws = list(range(w0, min(w0 + 4, NT)))
                            pw = psw.tile([128, 512], F32, tag="pw", name="pw")
                            for w in ws:
                                off = (w - w0) * 128
                                ops = []  # (half, tile_idx)
                                for half in (0, 1):
                                    Lpad, bounds, si, dl, bufs = halves[half]
                                    t0 = bounds[w] // 128
                                    t1 = (bounds[w + 1] - 1) // 128
                                    for t in range(t0, t1 + 1):
                                        ops.append((half, t))
                                for i, (half, t) in enumerate(ops):
                                    buf = get_chunk(half, t * 128 // CHUNK)
                                    slot = (t * 128 % CHUNK) // 128
                                    oc = opctr[0]
                                    opctr[0] += 1
                                    nc.tensor.matmul(
                                        pw[:, off:off + 128],
                                        buf[:, slot, :],
                                        sall[:, oc * 128:(oc + 1) * 128],
                                        start=(i == 0), stop=(i == len(ops) - 1),
                                        skip_group_check=True,
                                    )
                            gw = len(ws) * 128
                            blk = slice(w0 * 128, w0 * 128 + gw)
                            sadd = ep.tile([128, 512], F32, tag="sadd", name="sadd")
                            nc.vector.tensor_add(sadd[:, :gw], pw[:, :gw], dterm[:, blk])
                            nc.scalar.activation(out_fm[:, blk], sadd[:, :gw], AF.Relu)

                # ---------------- Layer 0 ----------------
                if PHASES < 3:
                    stk0.close()
                    stk1si.close()
                    return
                graph_layer(0, pre0)
                stk0.close()
                if PHASES < 4:
                    stk1si.close()
                    return
                # layer-1 sall streams in during md1 + AG1
                stk1 = ExitStack()
                gip1 = stk1.enter_context(tc.tile_pool(name=f"gi{rep}_1sall", bufs=1))
                pre1["sall"] = load_sall(1, gip1)
                # m1|d1 from out_fm; overwrite dterm with layer-1 dense term
                with (
                    tc.tile_pool(name=f"md1{rep}", bufs=4) as md1p,
                    tc.tile_pool(name=f"psC{rep}", bufs=3, space="PSUM") as psC,
                ):
                    for t0g in range(0, NT, 4):
                        tg = list(range(t0g, min(t0g + 4, NT)))
                        ps = psC.tile([128, 512], F32, tag="m1ps", name="m1ps")
                        for j, t in enumerate(tg):
                            lo = t * 128
                            nc.tensor.matmul(ps[:, j * 128:(j + 1) * 128],
                                             out_fm[:, lo:lo + 128], W["wl1"][:],
                                             skip_group_check=True)
                        gw = len(tg) * 128
                        m1t = md1p.tile([128, 512], BF16, tag="m1t", name="m1t")
                        nc.vector.tensor_copy(m1t[:, :gw], ps[:, :gw])
                        for j, t in enumerate(tg):
                            nc.sync.dma_start(out=m_own_dst(1, t),
                                              in_=m1t[:, j * 128:(j + 1) * 128])
                    allgather(1)
                    for lo, w in chunks512():
                        psd = psC.tile([D, 512], F32, tag="d1ps", name="d1ps")
                        nc.tensor.matmul(psd[:, :w], W["w011"][:], out_fm[:, lo:lo + w])
                        nc.scalar.activation(dterm[:, lo:lo + w], psd[:, :w],
                                             AF.Identity, bias=W["ball1"][:])

                # ---------------- Layer 1 ----------------
                if PHASES < 5:
                    stk1.close()
                    stk1si.close()
                    return
                graph_layer(1, pre1)
                if PHASES < 6:
                    stk1.close()
                    stk1si.close()
                    return

                # ---------------- Classifier + log_softmax ----------------
                # fc2/softmax in node-major: nodes on partitions, per-node
                # log-sum-exp via free-dim segmented reduce.
                with (
                    tc.tile_pool(name=f"fc{rep}", bufs=4) as fcp,
                    tc.tile_pool(name=f"fcb{rep}", bufs=1) as fcbp,
                    tc.tile_pool(name=f"psD{rep}", bufs=2, space="PSUM") as psD,
                ):
                    tfm = fcbp.tile([128, NPP], BF16, name="tfm")
                    for lo, w in chunks512():
                        ps = psD.tile([D, 512], F32, tag="fc1ps", name="fc1ps")
                        nc.tensor.matmul(ps[:, :w], W["fc1s"][:], out_fm[:, lo:lo + w])
                        nc.scalar.activation(tfm[:, lo:lo + w], ps[:, :w], AF.Relu,
                                             bias=W["fc1b"][:])
                    for g0 in range(0, NT, 12):
                        tg = list(range(g0, min(g0 + 12, NT)))
                        ng = len(tg)
                        gw = ng * NCLS
                        ps = psD.tile([128, 12 * NCLS], F32, tag="fc2ps", name="fc2ps")
                        for j, t in enumerate(tg):
                            nc.tensor.matmul(ps[:, j * NCLS:(j + 1) * NCLS],
                                             tfm[:, t * 128:(t + 1) * 128],
                                             W["fc2w"][:], skip_group_check=True)
                        lg = fcp.tile([128, 12 * NCLS], F32, tag="lg", name="lg")
                        nc.vector.tensor_add(lg[:, :gw], ps[:, :gw],
                                             W["fc2brep"][:, :gw])
                        ex = fcp.tile([128, 12 * NCLS], BF16, tag="ex", name="ex")
                        nc.scalar.activation(ex[:, :gw], lg[:, :gw], AF.Exp)
                        s = fcp.tile([128, 12], F32, tag="s", name="s")
                        nc.vector.reduce_sum(
                            s[:, :ng],
                            ex[:, :gw].rearrange("p (n c) -> p n c", c=NCLS),
                            axis=mybir.AxisListType.X)
                        lns = fcp.tile([128, 12], F32, tag="lns", name="lns")
                        nc.scalar.activation(lns[:, :ng], s[:, :ng], AF.Ln)
                        yt = fcp.tile([128, 12 * NCLS], F32, tag="yt", name="yt")
                        nc.vector.tensor_sub(
                            yt[:, :gw].rearrange("p (n c) -> p n c", c=NCLS),
                            lg[:, :gw].rearrange("p (n c) -> p n c", c=NCLS),
                            lns[:, :ng].rearrange("p (n o) -> p n o", o=1)
                               .to_broadcast([128, ng, NCLS]))
                        nc.sync.dma_start(
                            out=y_d[:, g0 * NCLS:g0 * NCLS + gw],
                            in_=yt[:, :gw])
                stk1.close()
                stk1si.close()
            for rep in range(REPEAT):
                body(rep)
    nc.compile()
    return nc


def kernel(**inputs):
    global LAST_EXEC_NS, LAST_RESULTS
    h = hashlib.md5()
    for k in sorted(inputs):
        h.update(np.ascontiguousarray(np.asarray(inputs[k])).tobytes())
    key = f"{REPEAT}{SKIP_AG}{SKIP_GATHER}{SKIP_GRAPH}{PREFETCH}{PHASES}" + h.hexdigest()
    if key not in _CACHE:
        in_maps, meta = _prep_inputs(inputs)
        nc = _build({k: (v[0], tuple(v[1])) for k, v in meta.items()})
        _CACHE[key] = (nc, in_maps)
    nc, in_maps = _CACHE[key]
    res = run_bass_kernel_spmd(nc, in_maps, list(range(P)), trace=TRACE, **TRACE_KW)
    LAST_EXEC_NS = res.exec_time_ns
    LAST_RESULTS = res
    outs = res.results
    y = np.concatenate(
        [outs[c]["y"].reshape(128, NT, NCLS).transpose(1, 0, 2)
         .reshape(NPP, NCLS)[:NPC] for c in range(P)],
        axis=0)
    return np.ascontiguousarray(y, dtype=np.float32)

